# revision 17
# baseline (speedup 1.0000x reference)
"""Trainium2 Bass kernel for ContextQuestionAttention (BiDAF-style).

Reference computation (per example):
    w1, w2, w3 = w[:H], w[H:2H], w[2H:]
    S[i,j] = C[i]·w1 + Q[j]·w2 + sum_h C[i,h] Q[j,h] w3[h]
    S = where(q_mask==0, -1e9, S)
    A = softmax_j(S) @ Q
    B_att = softmax_i(max_j S); B_vec = B_att @ C
    out = concat([C, A, C*A, C*B_vec], -1)

Sharding: data-parallel over batch, 4 examples per core on 8 cores.

The kernel is HBM-bound (9.4 MB in + 33.6 MB out per core @ ~360 GB/s =>
~117 us floor), so the layout aims at keeping the DMA queues saturated:

  - ALL input loads are issued up front (constants, then C of ex0, Q+mask,
    then C of ex1..3).  That fills the DMA pipe for the first ~26 us while
    compute ramps, and removes inter-example load/store serialization.
  - out[:, 0:H] = C verbatim: streamed SBUF->HBM from the loaded C tile via
    the ACT HWDGE ring (scalar.dma_start) so it neither blocks the SP ring
    nor costs an engine copy.  C*B_vec tiles go out on the Pool SWDGE ring.
    Per-i-tile [A|C*A] bursts go on the SP ring.  Three independent DMA
    issue streams -> no head-of-line blocking.
  - V^T[j,i] = s_cq^T + s_q[j] + maskbias[j] with j on partitions:
    matmul(lhsT=(w3*Q)^T chunks, rhs=C^T chunks) in PSUM; the per-partition
    (s_q + mask) bias is folded into the Exp activation that produces
    P'T = exp(V^T).  The s_c[i] term cancels in softmax_j and is left out.
  - Big matmuls run as float32r (1 cycle/row for free-dim >= 256 vs 4 for
    fp32) via AP bitcast - 4x PE time; tolerance dwarfs tf32-ish rounding.
  - A[i,:] = P'T[:,i].T @ Q / Z'[i] - P'T is already the lhsT the A-matmul
    needs; Z' via ones-column matmuls interleaved with the A matmuls.
    C*A is fused on DVE as (pa * 1/Z') * C straight out of PSUM
    (scalar_tensor_tensor), in parallel with ACT producing A itself.
  - The E/B path (rowmax transposes, s_c, B_att, B_vec) is deferred until
    after the A-phase so first outputs leave as early as possible:
    E = exp(maxS) = rowmax_j(P') * exp(s_c); B_vec^T chunks via N=1
    matmuls contracting i; broadcast via K=1 matmul; C*B_vec on GpSimd.
  - exp() without max subtraction is safe: |S| <~ 13 for these magnitudes.
"""

import os
import sys
from contextlib import ExitStack

import numpy as np

for _p in ("/opt/trn_rl_repo", "/root/.axon_site/_ro/trn_rl_repo"):
    if os.path.isdir(_p) and _p not in sys.path:
        sys.path.append(_p)

import concourse.bass as bass
import concourse.tile as tile
from concourse import bacc, mybir
from concourse.bass_utils import run_bass_kernel_spmd

F32 = mybir.dt.float32
F32R = mybir.dt.float32r
BF16 = mybir.dt.bfloat16
I32 = mybir.dt.int32
AX = mybir.AxisListType
ALU = mybir.AluOpType
ACTF = mybir.ActivationFunctionType
ts = bass.ts

N_CORES = 8
B_TOTAL = 32
B_PER_CORE = B_TOTAL // N_CORES  # 4
CLEN = 1024
QLEN = 128
H = 512
NT = CLEN // 128  # 8 i-tiles per example
KH = H // 128     # 4 h-chunks
NEG = -1.0e9


def _r(ap):
    """Reinterpret an fp32 AP as float32r for full-rate PE matmuls."""
    return ap.bitcast(F32R)


def _emit_loads(nc, pools, aps, rep):
    """All input DMAs for one rep, issued up front on the SP ring."""
    (c_pool, ct_pool, q_pool, pt_pool, sm_pool, scr_pool, a_pool, ot_pool,
     p_mm, p_sm, p_ty) = pools
    C_ap, Q_ap, M_ap, O_ap, consts = aps

    calls = []
    Qall = q_pool.tile([128, B_PER_CORE * H], F32, tag="qall", bufs=2)
    QallB = q_pool.tile([128, B_PER_CORE * H], BF16, tag="qallb", bufs=2)
    mall = sm_pool.tile([128, B_PER_CORE], I32, tag="mall", bufs=2)
    for b in range(B_PER_CORE):
        call = c_pool.tile([128, NT * H], F32, tag="call", bufs=B_PER_CORE,
                           name=f"call{b}_{rep}")
        chalf = C_ap[b].rearrange("(g t p) h -> g p t h", g=2, p=128)
        for g in range(2):
            nc.sync.dma_start(call[:, ts(g, NT * H // 2)], chalf[g])
            if b == 0 and g == 0:
                # small loads slot in right behind the first C half so
                # ex0 compute can start as soon as possible
                nc.sync.dma_start(
                    Qall[:].rearrange("p (b h) -> p b h", h=H),
                    Q_ap.rearrange("b p h -> p b h"))
                nc.sync.dma_start(mall[:], M_ap.rearrange("b p -> p b"))
        calls.append(call)
    nc.vector.tensor_copy(QallB[:], Qall[:])
    return calls, Qall, QallB, mall


def _emit_prep(nc, pools, aps, b, calls, Qall, QallB, mall):
    """Loads passthrough + everything up to P'T and Z'-free A inputs."""
    (c_pool, ct_pool, q_pool, pt_pool, sm_pool, scr_pool, a_pool, ot_pool,
     p_mm, p_sm, p_ty) = pools
    C_ap, Q_ap, M_ap, O_ap, consts = aps
    (ident, identB, ones_row, onesb, ones_col, ones_col_b, w3c,
     w1c, w1cB, W2b) = consts

    call = calls[b]
    Csb = [call[:, ts(t, H)] for t in range(NT)]
    Qsb = Qall[:, ts(b, H)]
    QsbB = QallB[:, ts(b, H)]

    # ---- stream C straight back out to out[:, 0:H] on the ACT ring ----
    for g in range(2):
        nc.scalar.dma_start(
            O_ap[b][ts(g, CLEN // 2), 0:H].rearrange("(t p) h -> p t h",
                                                     p=128),
            call[:, ts(g, NT * H // 2)].rearrange("p (t h) -> p t h", h=H))

    # ---- mask bias + s_q (per-partition over j) ----
    mskf = sm_pool.tile([128, 1], F32, tag="mskf", bufs=2)
    nc.vector.tensor_copy(mskf[:], mall[:, b:b + 1])
    mb = sm_pool.tile([128, 1], F32, tag="mb", bufs=2)
    # (mask - 1) * 1e9  -> 0 where mask==1, -1e9 where mask==0
    nc.vector.tensor_scalar(
        out=mb[:], in0=mskf[:], scalar1=1.0, scalar2=1.0e9,
        op0=ALU.subtract, op1=ALU.mult)
    scr = scr_pool.tile([128, H], F32, tag="scr", bufs=2)
    sq = sm_pool.tile([128, 1], F32, tag="sq", bufs=2)
    sqe = sm_pool.tile([128, 1], F32, tag="sqe", bufs=2)
    # sqe[j] = mb[j] + sum_h Q[j,h] * w2[h]
    nc.vector.tensor_mul(scr[:], Qsb, W2b[:])
    nc.vector.reduce_sum(sq[:], scr[:], axis=AX.X)
    nc.vector.tensor_add(sqe[:], sq[:], mb[:])

    # ---- (w3 * Q)^T chunks ----
    QW3T = q_pool.tile([128, H], BF16, tag="qw3t", bufs=2)
    for k in range(KH):
        pqt = p_sm.tile([128, 128], F32, tag="sm", bufs=2)
        nc.tensor.transpose(pqt[:], Qsb[:, ts(k, 128)], ident[:])
        nc.vector.tensor_scalar_mul(QW3T[:, ts(k, 128)], pqt[:], w3c[k][:])

    # ---- C^T chunks + S^T matmul + fused bias/exp -> P'T, per half ----
    CT = [ct_pool.tile([128, CLEN], BF16, tag=f"ct{k}", bufs=2,
                       name=f"ct{k}_{b}")
          for k in range(KH)]
    PT = pt_pool.tile([128, CLEN], BF16, tag="pt", bufs=2)
    for half in range(2):
        for k in range(KH):
            pct = p_mm.tile([128, 512], F32, tag="mm", bufs=4)
            for tt in range(4):
                t = half * 4 + tt
                nc.tensor.transpose(
                    pct[:, ts(tt, 128)], Csb[t][:, ts(k, 128)], ident[:])
            if k % 2 == 0:
                nc.scalar.copy(CT[k][:, ts(half, 512)], pct[:])
            else:
                nc.vector.tensor_copy(CT[k][:, ts(half, 512)], pct[:])
        pst = p_mm.tile([128, 512], F32, tag="mm", bufs=4)
        for k in range(KH):
            nc.tensor.matmul(
                pst[:], QW3T[:, ts(k, 128)], CT[k][:, ts(half, 512)],
                start=(k == 0), stop=(k == KH - 1))
        # P'T = exp(s_cq^T + s_q + maskbias)
        nc.scalar.activation(PT[:, ts(half, 512)], pst[:], ACTF.Exp,
                             bias=sqe[:], scale=1.0)

    return dict(b=b, call=call, Csb=Csb, Qsb=Qsb, QsbB=QsbB,
                CT=CT, PT=PT)


def _emit_outA(nc, pools, aps, st):
    (c_pool, ct_pool, q_pool, pt_pool, sm_pool, scr_pool, a_pool, ot_pool,
     p_mm, p_sm, p_ty) = pools
    C_ap, Q_ap, M_ap, O_ap, consts = aps
    (ident, identB, ones_row, onesb, ones_col, ones_col_b, w3c,
     w1c, w1cB, W2b) = consts
    b, Csb, QsbB, PT = st["b"], st["Csb"], st["QsbB"], st["PT"]

    # ---- A path per i-tile; stage [A|C*A] and DMA as one 4KB-row burst ----
    # Z' for 4 tiles batched per PSUM bank, interleaved with the A matmuls.
    RZP = sm_pool.tile([128, NT], F32, tag="rzp", bufs=2)
    for g in range(2):
        pzg = p_ty.tile([128, 4], F32, tag="tiny", bufs=2)
        for tt in range(4):
            t = g * 4 + tt
            nc.tensor.matmul(pzg[:, tt:tt + 1], PT[:, ts(t, 128)],
                             ones_col_b[:], start=True, stop=True)
        nc.vector.reciprocal(RZP[:, ts(g, 4)], pzg[:])
        for tt in range(4):
            t = g * 4 + tt
            pa = p_mm.tile([128, 512], F32, tag="mm", bufs=4)
            nc.tensor.matmul(pa[:], PT[:, ts(t, 128)], QsbB,
                             start=True, stop=True)
            ot = ot_pool.tile([128, 2 * H], F32, tag="ot", bufs=6)
            # A = pa / Z' on ACT; C*A = (pa / Z') * C fused on DVE from PSUM
            nc.scalar.mul(ot[:, 0:H], pa[:], RZP[:, t:t + 1])
            nc.vector.scalar_tensor_tensor(
                out=ot[:, H:2 * H], in0=pa[:], scalar=RZP[:, t:t + 1],
                in1=Csb[t][:], op0=ALU.mult, op1=ALU.mult)
            nc.sync.dma_start(O_ap[b, ts(t, 128), H:3 * H], ot[:])


def _emit_outB(nc, pools, aps, st):
    (c_pool, ct_pool, q_pool, pt_pool, sm_pool, scr_pool, a_pool, ot_pool,
     p_mm, p_sm, p_ty) = pools
    C_ap, Q_ap, M_ap, O_ap, consts = aps
    (ident, identB, ones_row, onesb, ones_col, ones_col_b, w3c,
     w1c, w1cB, W2b) = consts
    b, Csb, CT, PT = st["b"], st["Csb"], st["CT"], st["PT"]

    # ---- s_c columns on PE (reuses CT): SC[:, t] = C_t @ w1 ----
    # 8 sequential accumulation groups share one PSUM bank (one per column)
    SC = sm_pool.tile([128, NT], F32, tag="sc", bufs=2)
    psc8 = p_ty.tile([128, NT], F32, tag="tiny", bufs=2)
    for t in range(NT):
        for k in range(KH):
            nc.tensor.matmul(psc8[:, t:t + 1], CT[k][:, ts(t, 128)],
                             w1cB[k][:], start=(k == 0), stop=(k == KH - 1))
    nc.scalar.copy(SC[:], psc8[:])

    # ---- row max of P' natural (PE transposes, batched 4-per-psum-bank,
    # one segmented reduce per batch) ----
    MXE = sm_pool.tile([128, NT], F32, tag="mxe", bufs=2)
    for g in range(2):
        ppn = p_sm.tile([128, 512], BF16, tag="sm", bufs=2)
        for tt in range(4):
            t = g * 4 + tt
            nc.tensor.transpose(ppn[:, ts(tt, 128)], PT[:, ts(t, 128)],
                                identB[:])
        nc.vector.reduce_max(
            MXE[:, ts(g, 4)], ppn[:].rearrange("p (t x) -> p t x", x=128),
            axis=AX.X)

    # ---- E = exp(maxS) = rowmax(P') * exp(s_c) ----
    esc = sm_pool.tile([128, NT], F32, tag="esc", bufs=2)
    nc.scalar.activation(esc[:], SC[:], ACTF.Exp)
    E = sm_pool.tile([128, NT], F32, tag="e", bufs=2)
    nc.vector.tensor_mul(E[:], MXE[:], esc[:])

    # ---- B path: B_vec^T chunks via N=1 matmuls contracting i ----
    # 4 sequential accumulation groups (one per chunk column) in one bank
    pbt4 = p_ty.tile([128, KH], F32, tag="tiny", bufs=2)
    for k in range(KH):
        for t in range(NT):
            nc.tensor.matmul(pbt4[:, k:k + 1], Csb[t][:, ts(k, 128)],
                             E[:, t:t + 1], start=(t == 0),
                             stop=(t == NT - 1))
    btc = sm_pool.tile([128, KH], F32, tag="btc", bufs=2)
    nc.scalar.copy(btc[:], pbt4[:])
    Btrow = sm_pool.tile([1, H], BF16, tag="btrow", bufs=2)
    for k in range(KH):
        ptr = p_sm.tile([1, 128], F32, tag="sm", bufs=2)
        nc.tensor.transpose(ptr[:], btc[:, k:k + 1], ident[:])
        nc.scalar.copy(Btrow[:, ts(k, 128)], ptr[:])
    # Z2 = sum(E): free-dim reduce on DVE, partition reduce via one matmul
    rse = sm_pool.tile([128, 1], F32, tag="rse", bufs=2)
    nc.vector.reduce_sum(rse[:], E[:], axis=AX.X)
    pz2 = p_ty.tile([1, 1], F32, tag="tiny", bufs=2)
    nc.tensor.matmul(pz2[:], rse[:], ones_col, start=True, stop=True)
    z2sb = sm_pool.tile([1, 1], BF16, tag="z2", bufs=2)
    nc.scalar.copy(z2sb[:], pz2[:])
    # broadcast row -> all partitions with K=1 matmuls
    pbb = p_mm.tile([128, 512], F32, tag="mm", bufs=4)
    nc.tensor.matmul(pbb[:], onesb[:], Btrow[:], start=True, stop=True)
    pzb = p_ty.tile([128, 1], F32, tag="tiny", bufs=2)
    nc.tensor.matmul(pzb[:], onesb[:], z2sb[:], start=True, stop=True)
    rzb = sm_pool.tile([128, 1], F32, tag="rzb", bufs=2)
    nc.vector.reciprocal(rzb[:], pzb[:])
    Bb = a_pool.tile([128, H], F32, tag="bb", bufs=2)
    nc.scalar.mul(Bb[:], pbb[:], rzb[:])
    # C*B_vec tiles split GpSimd/DVE, staged per half-example; the DMAs
    # ride the Pool SWDGE ring (self-ordered behind the muls).
    for g in range(2):
        cb = a_pool.tile([128, NT * H // 2], F32, tag="cb", bufs=2)
        for tt in range(4):
            t = g * 4 + tt
            nc.gpsimd.tensor_mul(cb[:, ts(tt, H)], Csb[t][:], Bb[:])
        nc.gpsimd.dma_start(
            O_ap[b][ts(g, CLEN // 2), 3 * H:4 * H].rearrange(
                "(t p) h -> p t h", p=128),
            cb[:].rearrange("p (t h) -> p t h", h=H))


def build_nc(n_rep: int = 1):
    nc = bacc.Bacc("TRN2", target_bir_lowering=False, debug=False,
                   num_devices=N_CORES)
    C_ap = nc.dram_tensor("C", [B_PER_CORE, CLEN, H], F32,
                          kind="ExternalInput").ap()
    Q_ap = nc.dram_tensor("Q", [B_PER_CORE, QLEN, H], F32,
                          kind="ExternalInput").ap()
    M_ap = nc.dram_tensor("q_mask", [B_PER_CORE, QLEN], I32,
                          kind="ExternalInput").ap()
    W_ap = nc.dram_tensor("w", [3 * H], F32, kind="ExternalInput").ap()
    ID_ap = nc.dram_tensor("ident", [128, 128], F32,
                           kind="ExternalInput").ap()
    O_ap = nc.dram_tensor("out", [B_PER_CORE, CLEN, 4 * H], F32,
                          kind="ExternalOutput").ap()

    with tile.TileContext(nc) as tc, ExitStack() as ctx:
        const_pool = ctx.enter_context(tc.tile_pool(name="const", bufs=1))
        c_pool = ctx.enter_context(tc.tile_pool(name="cpool",
                                                bufs=B_PER_CORE))
        ct_pool = ctx.enter_context(tc.tile_pool(name="ctpool", bufs=2))
        q_pool = ctx.enter_context(tc.tile_pool(name="qpool", bufs=2))
        pt_pool = ctx.enter_context(tc.tile_pool(name="ptpool", bufs=2))
        sm_pool = ctx.enter_context(tc.tile_pool(name="smpool", bufs=2))
        scr_pool = ctx.enter_context(tc.tile_pool(name="scrpool", bufs=2))
        a_pool = ctx.enter_context(tc.tile_pool(name="apool", bufs=3))
        ot_pool = ctx.enter_context(tc.tile_pool(name="otpool", bufs=3))
        p_mm = ctx.enter_context(tc.tile_pool(name="pmm", bufs=4,
                                              space="PSUM"))
        p_sm = ctx.enter_context(tc.tile_pool(name="psm", bufs=2,
                                              space="PSUM"))
        p_ty = ctx.enter_context(tc.tile_pool(name="pty", bufs=2,
                                              space="PSUM"))

        # constants: ident + w as a single 6KB row (1 descriptor), then
        # w1/w3 columns via PE transposes of the row chunks
        ident = const_pool.tile([128, 128], F32, tag="ident")
        nc.sync.dma_start(ident[:], ID_ap[:])
        ones_row = const_pool.tile([1, 128], F32, tag="ones_row")
        nc.vector.memset(ones_row[:], 1.0)
        ones_col = nc.const_aps.tensor(1.0, (128, 1))
        wrow = const_pool.tile([1, 3 * H], F32, tag="wrow")
        nc.sync.dma_start(wrow[:], W_ap.rearrange("(a c) -> a c", a=1))
        wsb = const_pool.tile([128, 12], F32, tag="wsb")
        wsbB = const_pool.tile([128, 12], BF16, tag="wsbB")
        pwc = p_ty.tile([128, 12], F32, tag="tiny", bufs=2)
        for c in range(12):
            # [1,128] -> [128,1] transpose: K=1, so the "identity" is [1,1]
            nc.tensor.transpose(pwc[:, c:c + 1], wrow[:, ts(c, 128)],
                                ones_row[:, 0:1])
        nc.vector.tensor_copy(wsb[:], pwc[:])
        nc.vector.tensor_copy(wsbB[:], pwc[:])
        w1c = [wsb[:, k:k + 1] for k in range(KH)]
        w1cB = [wsbB[:, k:k + 1] for k in range(KH)]
        w3c = [wsb[:, 8 + k:9 + k] for k in range(KH)]
        identB = const_pool.tile([128, 128], BF16, tag="identB")
        nc.vector.tensor_copy(identB[:], ident[:])
        onesb = const_pool.tile([1, 128], BF16, tag="onesb")
        nc.vector.memset(onesb[:], 1.0)
        ones_col_b = const_pool.tile([128, 1], BF16, tag="onescolb")
        nc.vector.memset(ones_col_b[:], 1.0)
        # broadcast w2 across partitions via K=1 matmul
        W2b = const_pool.tile([128, H], F32, tag="w2b")
        pw = p_mm.tile([128, 512], F32, tag="mm", bufs=4)
        nc.tensor.matmul(pw[:], ones_row[:], wrow[:, H:2 * H],
                         start=True, stop=True)
        nc.vector.tensor_copy(W2b[:], pw[:])

        consts = (ident, identB, ones_row, onesb, ones_col,
                  ones_col_b, w3c, w1c, w1cB, W2b)
        pools = (c_pool, ct_pool, q_pool, pt_pool, sm_pool, scr_pool, a_pool,
                 ot_pool, p_mm, p_sm, p_ty)
        aps = (C_ap, Q_ap, M_ap, O_ap, consts)

        for rep in range(n_rep):
            calls, Qall, QallB, mall = _emit_loads(nc, pools, aps, rep)
            # software pipeline: emit prep(b+1) before outputs(b) so each
            # engine stream has next-example prep work queued behind the
            # current example's output work
            prev = None
            for b in range(B_PER_CORE):
                st = _emit_prep(nc, pools, aps, b, calls, Qall, QallB,
                                mall)
                if prev is not None:
                    _emit_outA(nc, pools, aps, prev)
                    _emit_outB(nc, pools, aps, prev)
                prev = st
            _emit_outA(nc, pools, aps, prev)
            _emit_outB(nc, pools, aps, prev)

    nc.compile()
    return nc


_NC_CACHE: dict = {}


def _get_nc(n_rep: int = 1):
    key = ("nc", n_rep)
    if key not in _NC_CACHE:
        _NC_CACHE[key] = build_nc(n_rep)
    return _NC_CACHE[key]


def make_in_maps(C, Q, q_mask, w):
    ident = np.eye(128, dtype=np.float32)
    w = np.ascontiguousarray(w, dtype=np.float32)
    in_maps = []
    for c in range(N_CORES):
        sl = slice(c * B_PER_CORE, (c + 1) * B_PER_CORE)
        in_maps.append({
            "C": np.ascontiguousarray(C[sl], dtype=np.float32),
            "Q": np.ascontiguousarray(Q[sl], dtype=np.float32),
            "q_mask": np.ascontiguousarray(q_mask[sl], dtype=np.int32),
            "w": w,
            "ident": ident,
        })
    return in_maps


def kernel(C, Q, q_mask, w):
    nc = _get_nc(1)
    in_maps = make_in_maps(C, Q, q_mask, w)
    res = run_bass_kernel_spmd(nc, in_maps, list(range(N_CORES)))
    out = np.concatenate([res.results[c]["out"] for c in range(N_CORES)],
                         axis=0)
    return out


# revision 18
# speedup vs baseline: 1.0258x; 1.0258x over previous
"""Trainium2 Bass kernel for ContextQuestionAttention (BiDAF-style).

Reference computation (per example):
    w1, w2, w3 = w[:H], w[H:2H], w[2H:]
    S[i,j] = C[i]·w1 + Q[j]·w2 + sum_h C[i,h] Q[j,h] w3[h]
    S = where(q_mask==0, -1e9, S)
    A = softmax_j(S) @ Q
    B_att = softmax_i(max_j S); B_vec = B_att @ C
    out = concat([C, A, C*A, C*B_vec], -1)

Sharding: data-parallel over batch, 4 examples per core on 8 cores.

The kernel is HBM-bound (9.4 MB in + 33.6 MB out per core @ ~360 GB/s =>
~117 us floor), so the layout aims at keeping the DMA queues saturated:

  - ALL input loads are issued up front (constants, then C of ex0, Q+mask,
    then C of ex1..3).  That fills the DMA pipe for the first ~26 us while
    compute ramps, and removes inter-example load/store serialization.
  - out[:, 0:H] = C verbatim: streamed SBUF->HBM from the loaded C tile via
    the ACT HWDGE ring (scalar.dma_start) so it neither blocks the SP ring
    nor costs an engine copy.  C*B_vec tiles go out on the Pool SWDGE ring.
    Per-i-tile [A|C*A] bursts go on the SP ring.  Three independent DMA
    issue streams -> no head-of-line blocking.
  - V^T[j,i] = s_cq^T + s_q[j] + maskbias[j] with j on partitions:
    matmul(lhsT=(w3*Q)^T chunks, rhs=C^T chunks) in PSUM; the per-partition
    (s_q + mask) bias is folded into the Exp activation that produces
    P'T = exp(V^T).  The s_c[i] term cancels in softmax_j and is left out.
  - Big matmuls run as float32r (1 cycle/row for free-dim >= 256 vs 4 for
    fp32) via AP bitcast - 4x PE time; tolerance dwarfs tf32-ish rounding.
  - A[i,:] = P'T[:,i].T @ Q / Z'[i] - P'T is already the lhsT the A-matmul
    needs; Z' via ones-column matmuls interleaved with the A matmuls.
    C*A is fused on DVE as (pa * 1/Z') * C straight out of PSUM
    (scalar_tensor_tensor), in parallel with ACT producing A itself.
  - The E/B path (rowmax transposes, s_c, B_att, B_vec) is deferred until
    after the A-phase so first outputs leave as early as possible:
    E = exp(maxS) = rowmax_j(P') * exp(s_c); B_vec^T chunks via N=1
    matmuls contracting i; broadcast via K=1 matmul; C*B_vec on GpSimd.
  - exp() without max subtraction is safe: |S| <~ 13 for these magnitudes.
"""

import os
import sys
from contextlib import ExitStack

import numpy as np

for _p in ("/opt/trn_rl_repo", "/root/.axon_site/_ro/trn_rl_repo"):
    if os.path.isdir(_p) and _p not in sys.path:
        sys.path.append(_p)

import concourse.bass as bass
import concourse.tile as tile
from concourse import bacc, mybir
from concourse.bass_utils import run_bass_kernel_spmd

F32 = mybir.dt.float32
F32R = mybir.dt.float32r
BF16 = mybir.dt.bfloat16
I32 = mybir.dt.int32
AX = mybir.AxisListType
ALU = mybir.AluOpType
ACTF = mybir.ActivationFunctionType
ts = bass.ts

N_CORES = 8
B_TOTAL = 32
B_PER_CORE = B_TOTAL // N_CORES  # 4
CLEN = 1024
QLEN = 128
H = 512
NT = CLEN // 128  # 8 i-tiles per example
KH = H // 128     # 4 h-chunks
NEG = -1.0e9


def _r(ap):
    """Reinterpret an fp32 AP as float32r for full-rate PE matmuls."""
    return ap.bitcast(F32R)


def _emit_loads(nc, pools, aps, rep):
    """All input DMAs for one rep, issued up front on the SP ring."""
    (c_pool, ct_pool, q_pool, pt_pool, sm_pool, scr_pool, a_pool, ot_pool,
     p_mm, p_sm, p_ty) = pools
    C_ap, Q_ap, M_ap, O_ap, consts = aps

    calls = []
    Qall = q_pool.tile([128, B_PER_CORE * H], F32, tag="qall", bufs=2)
    QallB = q_pool.tile([128, B_PER_CORE * H], BF16, tag="qallb", bufs=2)
    mall = sm_pool.tile([128, B_PER_CORE], I32, tag="mall", bufs=2)
    for b in range(B_PER_CORE):
        call = c_pool.tile([128, NT * H], F32, tag="call", bufs=B_PER_CORE,
                           name=f"call{b}_{rep}")
        chalf = C_ap[b].rearrange("(g t p) h -> g p t h", g=2, p=128)
        for g in range(2):
            nc.sync.dma_start(call[:, ts(g, NT * H // 2)], chalf[g])
            if b == 0 and g == 0:
                # small loads slot in right behind the first C half so
                # ex0 compute can start as soon as possible
                nc.sync.dma_start(
                    Qall[:].rearrange("p (b h) -> p b h", h=H),
                    Q_ap.rearrange("b p h -> p b h"))
                nc.sync.dma_start(mall[:], M_ap.rearrange("b p -> p b"))
        calls.append(call)
    nc.vector.tensor_copy(QallB[:], Qall[:])
    return calls, Qall, QallB, mall


def _emit_prep(nc, pools, aps, b, calls, Qall, QallB, mall):
    """Loads passthrough + everything up to P'T and Z'-free A inputs."""
    (c_pool, ct_pool, q_pool, pt_pool, sm_pool, scr_pool, a_pool, ot_pool,
     p_mm, p_sm, p_ty) = pools
    C_ap, Q_ap, M_ap, O_ap, consts = aps
    (ident, identB, ones_row, onesb, ones_col, ones_col_b, w3c,
     w1c, w1cB, W2b) = consts

    call = calls[b]
    Csb = [call[:, ts(t, H)] for t in range(NT)]
    Qsb = Qall[:, ts(b, H)]
    QsbB = QallB[:, ts(b, H)]

    # ---- stream C straight back out to out[:, 0:H] ----
    for g in range(2):
        nc.sync.dma_start(
            O_ap[b][ts(g, CLEN // 2), 0:H].rearrange("(t p) h -> p t h",
                                                     p=128),
            call[:, ts(g, NT * H // 2)].rearrange("p (t h) -> p t h", h=H))

    # ---- mask bias + s_q (per-partition over j) ----
    mskf = sm_pool.tile([128, 1], F32, tag="mskf", bufs=2)
    nc.vector.tensor_copy(mskf[:], mall[:, b:b + 1])
    mb = sm_pool.tile([128, 1], F32, tag="mb", bufs=2)
    # (mask - 1) * 1e9  -> 0 where mask==1, -1e9 where mask==0
    nc.vector.tensor_scalar(
        out=mb[:], in0=mskf[:], scalar1=1.0, scalar2=1.0e9,
        op0=ALU.subtract, op1=ALU.mult)
    scr = scr_pool.tile([128, H], F32, tag="scr", bufs=2)
    sq = sm_pool.tile([128, 1], F32, tag="sq", bufs=2)
    sqe = sm_pool.tile([128, 1], F32, tag="sqe", bufs=2)
    # sqe[j] = mb[j] + sum_h Q[j,h] * w2[h]
    nc.vector.tensor_mul(scr[:], Qsb, W2b[:])
    nc.vector.reduce_sum(sq[:], scr[:], axis=AX.X)
    nc.vector.tensor_add(sqe[:], sq[:], mb[:])

    # ---- (w3 * Q)^T chunks ----
    QW3T = q_pool.tile([128, H], BF16, tag="qw3t", bufs=2)
    for k in range(KH):
        pqt = p_sm.tile([128, 128], F32, tag="sm", bufs=2)
        nc.tensor.transpose(pqt[:], Qsb[:, ts(k, 128)], ident[:])
        nc.vector.tensor_scalar_mul(QW3T[:, ts(k, 128)], pqt[:], w3c[k][:])

    # ---- C^T chunks + S^T matmul + fused bias/exp -> P'T, per half ----
    CT = [ct_pool.tile([128, CLEN], BF16, tag=f"ct{k}", bufs=2,
                       name=f"ct{k}_{b}")
          for k in range(KH)]
    PT = pt_pool.tile([128, CLEN], BF16, tag="pt", bufs=2)
    for half in range(2):
        for k in range(KH):
            pct = p_mm.tile([128, 512], F32, tag="mm", bufs=4)
            for tt in range(4):
                t = half * 4 + tt
                nc.tensor.transpose(
                    pct[:, ts(tt, 128)], Csb[t][:, ts(k, 128)], ident[:])
            if k % 2 == 0:
                nc.scalar.copy(CT[k][:, ts(half, 512)], pct[:])
            else:
                nc.vector.tensor_copy(CT[k][:, ts(half, 512)], pct[:])
        pst = p_mm.tile([128, 512], F32, tag="mm", bufs=4)
        for k in range(KH):
            nc.tensor.matmul(
                pst[:], QW3T[:, ts(k, 128)], CT[k][:, ts(half, 512)],
                start=(k == 0), stop=(k == KH - 1))
        # P'T = exp(s_cq^T + s_q + maskbias)
        nc.scalar.activation(PT[:, ts(half, 512)], pst[:], ACTF.Exp,
                             bias=sqe[:], scale=1.0)

    return dict(b=b, call=call, Csb=Csb, Qsb=Qsb, QsbB=QsbB,
                CT=CT, PT=PT)


def _emit_outA(nc, pools, aps, st):
    (c_pool, ct_pool, q_pool, pt_pool, sm_pool, scr_pool, a_pool, ot_pool,
     p_mm, p_sm, p_ty) = pools
    C_ap, Q_ap, M_ap, O_ap, consts = aps
    (ident, identB, ones_row, onesb, ones_col, ones_col_b, w3c,
     w1c, w1cB, W2b) = consts
    b, Csb, QsbB, PT = st["b"], st["Csb"], st["QsbB"], st["PT"]

    # ---- A path per i-tile; stage [A|C*A] and DMA as one 4KB-row burst ----
    # Z' for 4 tiles batched per PSUM bank, interleaved with the A matmuls.
    RZP = sm_pool.tile([128, NT], F32, tag="rzp", bufs=2)
    for g in range(2):
        pzg = p_ty.tile([128, 4], F32, tag="tiny", bufs=2)
        for tt in range(4):
            t = g * 4 + tt
            nc.tensor.matmul(pzg[:, tt:tt + 1], PT[:, ts(t, 128)],
                             ones_col_b[:], start=True, stop=True)
        nc.vector.reciprocal(RZP[:, ts(g, 4)], pzg[:])
        for tt in range(4):
            t = g * 4 + tt
            pa = p_mm.tile([128, 512], F32, tag="mm", bufs=4)
            nc.tensor.matmul(pa[:], PT[:, ts(t, 128)], QsbB,
                             start=True, stop=True)
            ot = ot_pool.tile([128, 2 * H], F32, tag="ot", bufs=6)
            # A = pa / Z' on ACT; C*A = (pa / Z') * C fused on DVE from PSUM
            nc.scalar.mul(ot[:, 0:H], pa[:], RZP[:, t:t + 1])
            nc.vector.scalar_tensor_tensor(
                out=ot[:, H:2 * H], in0=pa[:], scalar=RZP[:, t:t + 1],
                in1=Csb[t][:], op0=ALU.mult, op1=ALU.mult)
            nc.sync.dma_start(O_ap[b, ts(t, 128), H:3 * H], ot[:])


def _emit_outB(nc, pools, aps, st):
    (c_pool, ct_pool, q_pool, pt_pool, sm_pool, scr_pool, a_pool, ot_pool,
     p_mm, p_sm, p_ty) = pools
    C_ap, Q_ap, M_ap, O_ap, consts = aps
    (ident, identB, ones_row, onesb, ones_col, ones_col_b, w3c,
     w1c, w1cB, W2b) = consts
    b, Csb, CT, PT = st["b"], st["Csb"], st["CT"], st["PT"]

    # ---- s_c columns on PE (reuses CT): SC[:, t] = C_t @ w1 ----
    # 8 sequential accumulation groups share one PSUM bank (one per column)
    SC = sm_pool.tile([128, NT], F32, tag="sc", bufs=2)
    psc8 = p_ty.tile([128, NT], F32, tag="tiny", bufs=2)
    for t in range(NT):
        for k in range(KH):
            nc.tensor.matmul(psc8[:, t:t + 1], CT[k][:, ts(t, 128)],
                             w1cB[k][:], start=(k == 0), stop=(k == KH - 1))
    nc.scalar.copy(SC[:], psc8[:])

    # ---- row max of P' natural (PE transposes, batched 4-per-psum-bank,
    # one segmented reduce per batch) ----
    MXE = sm_pool.tile([128, NT], F32, tag="mxe", bufs=2)
    for g in range(2):
        ppn = p_sm.tile([128, 512], BF16, tag="sm", bufs=2)
        for tt in range(4):
            t = g * 4 + tt
            nc.tensor.transpose(ppn[:, ts(tt, 128)], PT[:, ts(t, 128)],
                                identB[:])
        nc.vector.reduce_max(
            MXE[:, ts(g, 4)], ppn[:].rearrange("p (t x) -> p t x", x=128),
            axis=AX.X)

    # ---- E = exp(maxS) = rowmax(P') * exp(s_c) ----
    esc = sm_pool.tile([128, NT], F32, tag="esc", bufs=2)
    nc.scalar.activation(esc[:], SC[:], ACTF.Exp)
    E = sm_pool.tile([128, NT], F32, tag="e", bufs=2)
    nc.vector.tensor_mul(E[:], MXE[:], esc[:])

    # ---- B path: B_vec^T chunks via N=1 matmuls contracting i ----
    # 4 sequential accumulation groups (one per chunk column) in one bank
    pbt4 = p_ty.tile([128, KH], F32, tag="tiny", bufs=2)
    for k in range(KH):
        for t in range(NT):
            nc.tensor.matmul(pbt4[:, k:k + 1], Csb[t][:, ts(k, 128)],
                             E[:, t:t + 1], start=(t == 0),
                             stop=(t == NT - 1))
    btc = sm_pool.tile([128, KH], F32, tag="btc", bufs=2)
    nc.scalar.copy(btc[:], pbt4[:])
    Btrow = sm_pool.tile([1, H], BF16, tag="btrow", bufs=2)
    for k in range(KH):
        ptr = p_sm.tile([1, 128], F32, tag="sm", bufs=2)
        nc.tensor.transpose(ptr[:], btc[:, k:k + 1], ident[:])
        nc.scalar.copy(Btrow[:, ts(k, 128)], ptr[:])
    # Z2 = sum(E): free-dim reduce on DVE, partition reduce via one matmul
    rse = sm_pool.tile([128, 1], F32, tag="rse", bufs=2)
    nc.vector.reduce_sum(rse[:], E[:], axis=AX.X)
    pz2 = p_ty.tile([1, 1], F32, tag="tiny", bufs=2)
    nc.tensor.matmul(pz2[:], rse[:], ones_col, start=True, stop=True)
    z2sb = sm_pool.tile([1, 1], BF16, tag="z2", bufs=2)
    nc.scalar.copy(z2sb[:], pz2[:])
    # broadcast row -> all partitions with K=1 matmuls
    pbb = p_mm.tile([128, 512], F32, tag="mm", bufs=4)
    nc.tensor.matmul(pbb[:], onesb[:], Btrow[:], start=True, stop=True)
    pzb = p_ty.tile([128, 1], F32, tag="tiny", bufs=2)
    nc.tensor.matmul(pzb[:], onesb[:], z2sb[:], start=True, stop=True)
    rzb = sm_pool.tile([128, 1], F32, tag="rzb", bufs=2)
    nc.vector.reciprocal(rzb[:], pzb[:])
    Bb = a_pool.tile([128, H], F32, tag="bb", bufs=2)
    nc.scalar.mul(Bb[:], pbb[:], rzb[:])
    # C*B_vec tiles split GpSimd/DVE, staged per half-example; the DMAs
    # ride the Pool SWDGE ring (self-ordered behind the muls).
    for g in range(2):
        cb = a_pool.tile([128, NT * H // 2], F32, tag="cb", bufs=2)
        for tt in range(4):
            t = g * 4 + tt
            nc.gpsimd.tensor_mul(cb[:, ts(tt, H)], Csb[t][:], Bb[:])
        nc.sync.dma_start(
            O_ap[b][ts(g, CLEN // 2), 3 * H:4 * H].rearrange(
                "(t p) h -> p t h", p=128),
            cb[:].rearrange("p (t h) -> p t h", h=H))


def build_nc(n_rep: int = 1):
    nc = bacc.Bacc("TRN2", target_bir_lowering=False, debug=False,
                   num_devices=N_CORES)
    C_ap = nc.dram_tensor("C", [B_PER_CORE, CLEN, H], F32,
                          kind="ExternalInput").ap()
    Q_ap = nc.dram_tensor("Q", [B_PER_CORE, QLEN, H], F32,
                          kind="ExternalInput").ap()
    M_ap = nc.dram_tensor("q_mask", [B_PER_CORE, QLEN], I32,
                          kind="ExternalInput").ap()
    W_ap = nc.dram_tensor("w", [3 * H], F32, kind="ExternalInput").ap()
    ID_ap = nc.dram_tensor("ident", [128, 128], F32,
                           kind="ExternalInput").ap()
    O_ap = nc.dram_tensor("out", [B_PER_CORE, CLEN, 4 * H], F32,
                          kind="ExternalOutput").ap()

    with tile.TileContext(nc) as tc, ExitStack() as ctx:
        const_pool = ctx.enter_context(tc.tile_pool(name="const", bufs=1))
        c_pool = ctx.enter_context(tc.tile_pool(name="cpool",
                                                bufs=B_PER_CORE))
        ct_pool = ctx.enter_context(tc.tile_pool(name="ctpool", bufs=2))
        q_pool = ctx.enter_context(tc.tile_pool(name="qpool", bufs=2))
        pt_pool = ctx.enter_context(tc.tile_pool(name="ptpool", bufs=2))
        sm_pool = ctx.enter_context(tc.tile_pool(name="smpool", bufs=2))
        scr_pool = ctx.enter_context(tc.tile_pool(name="scrpool", bufs=2))
        a_pool = ctx.enter_context(tc.tile_pool(name="apool", bufs=3))
        ot_pool = ctx.enter_context(tc.tile_pool(name="otpool", bufs=3))
        p_mm = ctx.enter_context(tc.tile_pool(name="pmm", bufs=4,
                                              space="PSUM"))
        p_sm = ctx.enter_context(tc.tile_pool(name="psm", bufs=2,
                                              space="PSUM"))
        p_ty = ctx.enter_context(tc.tile_pool(name="pty", bufs=2,
                                              space="PSUM"))

        # constants: ident + w as a single 6KB row (1 descriptor), then
        # w1/w3 columns via PE transposes of the row chunks
        ident = const_pool.tile([128, 128], F32, tag="ident")
        nc.sync.dma_start(ident[:], ID_ap[:])
        ones_row = const_pool.tile([1, 128], F32, tag="ones_row")
        nc.vector.memset(ones_row[:], 1.0)
        ones_col = nc.const_aps.tensor(1.0, (128, 1))
        wrow = const_pool.tile([1, 3 * H], F32, tag="wrow")
        nc.sync.dma_start(wrow[:], W_ap.rearrange("(a c) -> a c", a=1))
        wsb = const_pool.tile([128, 12], F32, tag="wsb")
        wsbB = const_pool.tile([128, 12], BF16, tag="wsbB")
        pwc = p_ty.tile([128, 12], F32, tag="tiny", bufs=2)
        for c in range(12):
            # [1,128] -> [128,1] transpose: K=1, so the "identity" is [1,1]
            nc.tensor.transpose(pwc[:, c:c + 1], wrow[:, ts(c, 128)],
                                ones_row[:, 0:1])
        nc.vector.tensor_copy(wsb[:], pwc[:])
        nc.vector.tensor_copy(wsbB[:], pwc[:])
        w1c = [wsb[:, k:k + 1] for k in range(KH)]
        w1cB = [wsbB[:, k:k + 1] for k in range(KH)]
        w3c = [wsb[:, 8 + k:9 + k] for k in range(KH)]
        identB = const_pool.tile([128, 128], BF16, tag="identB")
        nc.vector.tensor_copy(identB[:], ident[:])
        onesb = const_pool.tile([1, 128], BF16, tag="onesb")
        nc.vector.memset(onesb[:], 1.0)
        ones_col_b = const_pool.tile([128, 1], BF16, tag="onescolb")
        nc.vector.memset(ones_col_b[:], 1.0)
        # broadcast w2 across partitions via K=1 matmul
        W2b = const_pool.tile([128, H], F32, tag="w2b")
        pw = p_mm.tile([128, 512], F32, tag="mm", bufs=4)
        nc.tensor.matmul(pw[:], ones_row[:], wrow[:, H:2 * H],
                         start=True, stop=True)
        nc.vector.tensor_copy(W2b[:], pw[:])

        consts = (ident, identB, ones_row, onesb, ones_col,
                  ones_col_b, w3c, w1c, w1cB, W2b)
        pools = (c_pool, ct_pool, q_pool, pt_pool, sm_pool, scr_pool, a_pool,
                 ot_pool, p_mm, p_sm, p_ty)
        aps = (C_ap, Q_ap, M_ap, O_ap, consts)

        for rep in range(n_rep):
            calls, Qall, QallB, mall = _emit_loads(nc, pools, aps, rep)
            # software pipeline: emit prep(b+1) before outputs(b) so each
            # engine stream has next-example prep work queued behind the
            # current example's output work
            prev = None
            for b in range(B_PER_CORE):
                st = _emit_prep(nc, pools, aps, b, calls, Qall, QallB,
                                mall)
                if prev is not None:
                    _emit_outA(nc, pools, aps, prev)
                    _emit_outB(nc, pools, aps, prev)
                prev = st
            _emit_outA(nc, pools, aps, prev)
            _emit_outB(nc, pools, aps, prev)

    nc.compile()
    return nc


_NC_CACHE: dict = {}


def _get_nc(n_rep: int = 1):
    key = ("nc", n_rep)
    if key not in _NC_CACHE:
        _NC_CACHE[key] = build_nc(n_rep)
    return _NC_CACHE[key]


def make_in_maps(C, Q, q_mask, w):
    ident = np.eye(128, dtype=np.float32)
    w = np.ascontiguousarray(w, dtype=np.float32)
    in_maps = []
    for c in range(N_CORES):
        sl = slice(c * B_PER_CORE, (c + 1) * B_PER_CORE)
        in_maps.append({
            "C": np.ascontiguousarray(C[sl], dtype=np.float32),
            "Q": np.ascontiguousarray(Q[sl], dtype=np.float32),
            "q_mask": np.ascontiguousarray(q_mask[sl], dtype=np.int32),
            "w": w,
            "ident": ident,
        })
    return in_maps


def kernel(C, Q, q_mask, w):
    nc = _get_nc(1)
    in_maps = make_in_maps(C, Q, q_mask, w)
    res = run_bass_kernel_spmd(nc, in_maps, list(range(N_CORES)))
    out = np.concatenate([res.results[c]["out"] for c in range(N_CORES)],
                         axis=0)
    return out


# revision 19
# speedup vs baseline: 1.3936x; 1.3585x over previous
"""Trainium2 Bass kernel for ContextQuestionAttention (BiDAF-style).

Reference computation (per example):
    w1, w2, w3 = w[:H], w[H:2H], w[2H:]
    S[i,j] = C[i]·w1 + Q[j]·w2 + sum_h C[i,h] Q[j,h] w3[h]
    S = where(q_mask==0, -1e9, S)
    A = softmax_j(S) @ Q
    B_att = softmax_i(max_j S); B_vec = B_att @ C
    out = concat([C, A, C*A, C*B_vec], -1)

Sharding: data-parallel over batch, 4 examples per core on 8 cores.

The kernel is HBM-bound (9.4 MB in + 33.6 MB out per core @ ~360 GB/s =>
~117 us floor), so the layout aims at keeping the DMA queues saturated:

  - ALL input loads are issued up front (constants, then C of ex0, Q+mask,
    then C of ex1..3).  That fills the DMA pipe for the first ~26 us while
    compute ramps, and removes inter-example load/store serialization.
  - out[:, 0:H] = C verbatim: streamed SBUF->HBM from the loaded C tile via
    the ACT HWDGE ring (scalar.dma_start) so it neither blocks the SP ring
    nor costs an engine copy.  C*B_vec tiles go out on the Pool SWDGE ring.
    Per-i-tile [A|C*A] bursts go on the SP ring.  Three independent DMA
    issue streams -> no head-of-line blocking.
  - V^T[j,i] = s_cq^T + s_q[j] + maskbias[j] with j on partitions:
    matmul(lhsT=(w3*Q)^T chunks, rhs=C^T chunks) in PSUM; the per-partition
    (s_q + mask) bias is folded into the Exp activation that produces
    P'T = exp(V^T).  The s_c[i] term cancels in softmax_j and is left out.
  - Big matmuls run as float32r (1 cycle/row for free-dim >= 256 vs 4 for
    fp32) via AP bitcast - 4x PE time; tolerance dwarfs tf32-ish rounding.
  - A[i,:] = P'T[:,i].T @ Q / Z'[i] - P'T is already the lhsT the A-matmul
    needs; Z' via ones-column matmuls interleaved with the A matmuls.
    C*A is fused on DVE as (pa * 1/Z') * C straight out of PSUM
    (scalar_tensor_tensor), in parallel with ACT producing A itself.
  - The E/B path (rowmax transposes, s_c, B_att, B_vec) is deferred until
    after the A-phase so first outputs leave as early as possible:
    E = exp(maxS) = rowmax_j(P') * exp(s_c); B_vec^T chunks via N=1
    matmuls contracting i; broadcast via K=1 matmul; C*B_vec on GpSimd.
  - exp() without max subtraction is safe: |S| <~ 13 for these magnitudes.
"""

import os
import sys
from contextlib import ExitStack

import numpy as np

for _p in ("/opt/trn_rl_repo", "/root/.axon_site/_ro/trn_rl_repo"):
    if os.path.isdir(_p) and _p not in sys.path:
        sys.path.append(_p)

import concourse.bass as bass
import concourse.tile as tile
from concourse import bacc, mybir
from concourse.bass_utils import run_bass_kernel_spmd

F32 = mybir.dt.float32
F32R = mybir.dt.float32r
BF16 = mybir.dt.bfloat16
I32 = mybir.dt.int32
AX = mybir.AxisListType
ALU = mybir.AluOpType
ACTF = mybir.ActivationFunctionType
ts = bass.ts

N_CORES = 8
B_TOTAL = 32
B_PER_CORE = B_TOTAL // N_CORES  # 4
CLEN = 1024
QLEN = 128
H = 512
NT = CLEN // 128  # 8 i-tiles per example
KH = H // 128     # 4 h-chunks
NEG = -1.0e9


def _r(ap):
    """Reinterpret an fp32 AP as float32r for full-rate PE matmuls."""
    return ap.bitcast(F32R)


def _emit_load_ex(nc, pools, aps, i, rep, b, qstate):
    """Input DMAs for global example index i (prefetched ~2 ahead).
    Per-rep Q/mask batch loads ride with example b==0."""
    (c_pool, ct_pool, q_pool, pt_pool, sm_pool, scr_pool, a_pool, ot_pool,
     p_mm, p_sm, p_ty) = pools
    C_ap, Q_ap, M_ap, O_ap, consts = aps

    if b == 0:
        Qall = q_pool.tile([128, B_PER_CORE * H], F32, tag="qall", bufs=2)
        QallB = q_pool.tile([128, B_PER_CORE * H], BF16, tag="qallb", bufs=2)
        mall = sm_pool.tile([128, B_PER_CORE], I32, tag="mall", bufs=2)
        nc.sync.dma_start(
            Qall[:].rearrange("p (b h) -> p b h", h=H),
            Q_ap.rearrange("b p h -> p b h"))
        nc.sync.dma_start(mall[:], M_ap.rearrange("b p -> p b"))
        nc.vector.tensor_copy(QallB[:], Qall[:])
        qstate[rep] = (Qall, QallB, mall)
    call = c_pool.tile([128, NT * H], F32, tag="call", bufs=4,
                       name=f"call_{i}")
    chalf = C_ap[b].rearrange("(g t p) h -> g p t h", g=2, p=128)
    for g in range(2):
        nc.sync.dma_start(call[:, ts(g, NT * H // 2)], chalf[g])
    return call


def _emit_prep(nc, pools, aps, b, call, Qall, QallB, mall):
    """Loads passthrough + everything up to P'T and Z'-free A inputs."""
    (c_pool, ct_pool, q_pool, pt_pool, sm_pool, scr_pool, a_pool, ot_pool,
     p_mm, p_sm, p_ty) = pools
    C_ap, Q_ap, M_ap, O_ap, consts = aps
    (ident, identB, ones_row, onesb, ones_col, ones_col_b, w3c,
     w1c, w1cB, W2b) = consts

    Csb = [call[:, ts(t, H)] for t in range(NT)]
    Qsb = Qall[:, ts(b, H)]
    QsbB = QallB[:, ts(b, H)]

    # ---- stream C straight back out to out[:, 0:H] ----
    for g in range(2):
        nc.sync.dma_start(
            O_ap[b][ts(g, CLEN // 2), 0:H].rearrange("(t p) h -> p t h",
                                                     p=128),
            call[:, ts(g, NT * H // 2)].rearrange("p (t h) -> p t h", h=H))

    # ---- mask bias + s_q (per-partition over j) ----
    mskf = sm_pool.tile([128, 1], F32, tag="mskf", bufs=2)
    nc.vector.tensor_copy(mskf[:], mall[:, b:b + 1])
    mb = sm_pool.tile([128, 1], F32, tag="mb", bufs=2)
    # (mask - 1) * 1e9  -> 0 where mask==1, -1e9 where mask==0
    nc.vector.tensor_scalar(
        out=mb[:], in0=mskf[:], scalar1=1.0, scalar2=1.0e9,
        op0=ALU.subtract, op1=ALU.mult)
    scr = scr_pool.tile([128, H], F32, tag="scr", bufs=2)
    sq = sm_pool.tile([128, 1], F32, tag="sq", bufs=2)
    sqe = sm_pool.tile([128, 1], F32, tag="sqe", bufs=2)
    # sqe[j] = mb[j] + sum_h Q[j,h] * w2[h]
    nc.vector.tensor_mul(scr[:], Qsb, W2b[:])
    nc.vector.reduce_sum(sq[:], scr[:], axis=AX.X)
    nc.vector.tensor_add(sqe[:], sq[:], mb[:])

    # ---- (w3 * Q)^T chunks ----
    QW3T = q_pool.tile([128, H], BF16, tag="qw3t", bufs=2)
    for k in range(KH):
        pqt = p_sm.tile([128, 128], F32, tag="sm", bufs=2)
        nc.tensor.transpose(pqt[:], Qsb[:, ts(k, 128)], ident[:])
        nc.vector.tensor_scalar_mul(QW3T[:, ts(k, 128)], pqt[:], w3c[k][:])

    # ---- C^T chunks + S^T matmul + fused bias/exp -> P'T, per half ----
    CT = [ct_pool.tile([128, CLEN], BF16, tag=f"ct{k}", bufs=2,
                       name=f"ct{k}_{b}")
          for k in range(KH)]
    PT = pt_pool.tile([128, CLEN], BF16, tag="pt", bufs=2)
    for half in range(2):
        for k in range(KH):
            pct = p_mm.tile([128, 512], F32, tag="mm", bufs=4)
            for tt in range(4):
                t = half * 4 + tt
                nc.tensor.transpose(
                    pct[:, ts(tt, 128)], Csb[t][:, ts(k, 128)], ident[:])
            if k % 2 == 0:
                nc.scalar.copy(CT[k][:, ts(half, 512)], pct[:])
            else:
                nc.vector.tensor_copy(CT[k][:, ts(half, 512)], pct[:])
        pst = p_mm.tile([128, 512], F32, tag="mm", bufs=4)
        for k in range(KH):
            nc.tensor.matmul(
                pst[:], QW3T[:, ts(k, 128)], CT[k][:, ts(half, 512)],
                start=(k == 0), stop=(k == KH - 1))
        # P'T = exp(s_cq^T + s_q + maskbias)
        nc.scalar.activation(PT[:, ts(half, 512)], pst[:], ACTF.Exp,
                             bias=sqe[:], scale=1.0)

    return dict(b=b, call=call, Csb=Csb, Qsb=Qsb, QsbB=QsbB,
                CT=CT, PT=PT)


def _emit_outA(nc, pools, aps, st):
    (c_pool, ct_pool, q_pool, pt_pool, sm_pool, scr_pool, a_pool, ot_pool,
     p_mm, p_sm, p_ty) = pools
    C_ap, Q_ap, M_ap, O_ap, consts = aps
    (ident, identB, ones_row, onesb, ones_col, ones_col_b, w3c,
     w1c, w1cB, W2b) = consts
    b, Csb, QsbB, PT = st["b"], st["Csb"], st["QsbB"], st["PT"]

    # ---- A path per i-tile; stage [A|C*A] and DMA as one 4KB-row burst ----
    # Z' for 4 tiles batched per PSUM bank, interleaved with the A matmuls.
    RZP = sm_pool.tile([128, NT], F32, tag="rzp", bufs=2)
    for g in range(2):
        pzg = p_ty.tile([128, 4], F32, tag="tiny", bufs=2)
        for tt in range(4):
            t = g * 4 + tt
            nc.tensor.matmul(pzg[:, tt:tt + 1], PT[:, ts(t, 128)],
                             ones_col_b[:], start=True, stop=True)
        nc.vector.reciprocal(RZP[:, ts(g, 4)], pzg[:])
        for tt in range(4):
            t = g * 4 + tt
            pa = p_mm.tile([128, 512], F32, tag="mm", bufs=4)
            nc.tensor.matmul(pa[:], PT[:, ts(t, 128)], QsbB,
                             start=True, stop=True)
            ot = ot_pool.tile([128, 2 * H], F32, tag="ot", bufs=6)
            # A = pa / Z' on ACT; C*A = (pa / Z') * C fused on DVE from PSUM
            nc.scalar.mul(ot[:, 0:H], pa[:], RZP[:, t:t + 1])
            nc.vector.scalar_tensor_tensor(
                out=ot[:, H:2 * H], in0=pa[:], scalar=RZP[:, t:t + 1],
                in1=Csb[t][:], op0=ALU.mult, op1=ALU.mult)
            nc.sync.dma_start(O_ap[b, ts(t, 128), H:3 * H], ot[:])


def _emit_outB(nc, pools, aps, st):
    (c_pool, ct_pool, q_pool, pt_pool, sm_pool, scr_pool, a_pool, ot_pool,
     p_mm, p_sm, p_ty) = pools
    C_ap, Q_ap, M_ap, O_ap, consts = aps
    (ident, identB, ones_row, onesb, ones_col, ones_col_b, w3c,
     w1c, w1cB, W2b) = consts
    b, Csb, CT, PT = st["b"], st["Csb"], st["CT"], st["PT"]

    # ---- s_c columns on PE (reuses CT): SC[:, t] = C_t @ w1 ----
    # 8 sequential accumulation groups share one PSUM bank (one per column)
    SC = sm_pool.tile([128, NT], F32, tag="sc", bufs=2)
    psc8 = p_ty.tile([128, NT], F32, tag="tiny", bufs=2)
    for t in range(NT):
        for k in range(KH):
            nc.tensor.matmul(psc8[:, t:t + 1], CT[k][:, ts(t, 128)],
                             w1cB[k][:], start=(k == 0), stop=(k == KH - 1))
    nc.scalar.copy(SC[:], psc8[:])

    # ---- row max of P' natural (PE transposes, batched 4-per-psum-bank,
    # one segmented reduce per batch) ----
    MXE = sm_pool.tile([128, NT], F32, tag="mxe", bufs=2)
    for g in range(2):
        ppn = p_sm.tile([128, 512], BF16, tag="sm", bufs=2)
        for tt in range(4):
            t = g * 4 + tt
            nc.tensor.transpose(ppn[:, ts(tt, 128)], PT[:, ts(t, 128)],
                                identB[:])
        nc.vector.reduce_max(
            MXE[:, ts(g, 4)], ppn[:].rearrange("p (t x) -> p t x", x=128),
            axis=AX.X)

    # ---- E = exp(maxS) = rowmax(P') * exp(s_c) ----
    esc = sm_pool.tile([128, NT], F32, tag="esc", bufs=2)
    nc.scalar.activation(esc[:], SC[:], ACTF.Exp)
    E = sm_pool.tile([128, NT], F32, tag="e", bufs=2)
    nc.vector.tensor_mul(E[:], MXE[:], esc[:])

    # ---- B path: B_vec^T chunks via N=1 matmuls contracting i ----
    # 4 sequential accumulation groups (one per chunk column) in one bank
    pbt4 = p_ty.tile([128, KH], F32, tag="tiny", bufs=2)
    for k in range(KH):
        for t in range(NT):
            nc.tensor.matmul(pbt4[:, k:k + 1], Csb[t][:, ts(k, 128)],
                             E[:, t:t + 1], start=(t == 0),
                             stop=(t == NT - 1))
    btc = sm_pool.tile([128, KH], F32, tag="btc", bufs=2)
    nc.scalar.copy(btc[:], pbt4[:])
    Btrow = sm_pool.tile([1, H], BF16, tag="btrow", bufs=2)
    for k in range(KH):
        ptr = p_sm.tile([1, 128], F32, tag="sm", bufs=2)
        nc.tensor.transpose(ptr[:], btc[:, k:k + 1], ident[:])
        nc.scalar.copy(Btrow[:, ts(k, 128)], ptr[:])
    # Z2 = sum(E): free-dim reduce on DVE, partition reduce via one matmul
    rse = sm_pool.tile([128, 1], F32, tag="rse", bufs=2)
    nc.vector.reduce_sum(rse[:], E[:], axis=AX.X)
    pz2 = p_ty.tile([1, 1], F32, tag="tiny", bufs=2)
    nc.tensor.matmul(pz2[:], rse[:], ones_col, start=True, stop=True)
    z2sb = sm_pool.tile([1, 1], BF16, tag="z2", bufs=2)
    nc.scalar.copy(z2sb[:], pz2[:])
    # broadcast row -> all partitions with K=1 matmuls
    pbb = p_mm.tile([128, 512], F32, tag="mm", bufs=4)
    nc.tensor.matmul(pbb[:], onesb[:], Btrow[:], start=True, stop=True)
    pzb = p_ty.tile([128, 1], F32, tag="tiny", bufs=2)
    nc.tensor.matmul(pzb[:], onesb[:], z2sb[:], start=True, stop=True)
    rzb = sm_pool.tile([128, 1], F32, tag="rzb", bufs=2)
    nc.vector.reciprocal(rzb[:], pzb[:])
    Bb = a_pool.tile([128, H], F32, tag="bb", bufs=2)
    nc.scalar.mul(Bb[:], pbb[:], rzb[:])
    # C*B_vec tiles split GpSimd/DVE, staged per half-example; the DMAs
    # ride the Pool SWDGE ring (self-ordered behind the muls).
    for g in range(2):
        cb = a_pool.tile([128, NT * H // 2], F32, tag="cb", bufs=2)
        for tt in range(4):
            t = g * 4 + tt
            nc.gpsimd.tensor_mul(cb[:, ts(tt, H)], Csb[t][:], Bb[:])
        nc.sync.dma_start(
            O_ap[b][ts(g, CLEN // 2), 3 * H:4 * H].rearrange(
                "(t p) h -> p t h", p=128),
            cb[:].rearrange("p (t h) -> p t h", h=H))


def build_nc(n_rep: int = 1):
    nc = bacc.Bacc("TRN2", target_bir_lowering=False, debug=False,
                   num_devices=N_CORES)
    C_ap = nc.dram_tensor("C", [B_PER_CORE, CLEN, H], F32,
                          kind="ExternalInput").ap()
    Q_ap = nc.dram_tensor("Q", [B_PER_CORE, QLEN, H], F32,
                          kind="ExternalInput").ap()
    M_ap = nc.dram_tensor("q_mask", [B_PER_CORE, QLEN], I32,
                          kind="ExternalInput").ap()
    W_ap = nc.dram_tensor("w", [3 * H], F32, kind="ExternalInput").ap()
    ID_ap = nc.dram_tensor("ident", [128, 128], F32,
                           kind="ExternalInput").ap()
    O_ap = nc.dram_tensor("out", [B_PER_CORE, CLEN, 4 * H], F32,
                          kind="ExternalOutput").ap()

    with tile.TileContext(nc) as tc, ExitStack() as ctx:
        const_pool = ctx.enter_context(tc.tile_pool(name="const", bufs=1))
        c_pool = ctx.enter_context(tc.tile_pool(name="cpool",
                                                bufs=B_PER_CORE))
        ct_pool = ctx.enter_context(tc.tile_pool(name="ctpool", bufs=2))
        q_pool = ctx.enter_context(tc.tile_pool(name="qpool", bufs=2))
        pt_pool = ctx.enter_context(tc.tile_pool(name="ptpool", bufs=2))
        sm_pool = ctx.enter_context(tc.tile_pool(name="smpool", bufs=2))
        scr_pool = ctx.enter_context(tc.tile_pool(name="scrpool", bufs=2))
        a_pool = ctx.enter_context(tc.tile_pool(name="apool", bufs=3))
        ot_pool = ctx.enter_context(tc.tile_pool(name="otpool", bufs=3))
        p_mm = ctx.enter_context(tc.tile_pool(name="pmm", bufs=4,
                                              space="PSUM"))
        p_sm = ctx.enter_context(tc.tile_pool(name="psm", bufs=2,
                                              space="PSUM"))
        p_ty = ctx.enter_context(tc.tile_pool(name="pty", bufs=2,
                                              space="PSUM"))

        # constants: ident + w as a single 6KB row (1 descriptor), then
        # w1/w3 columns via PE transposes of the row chunks
        ident = const_pool.tile([128, 128], F32, tag="ident")
        nc.sync.dma_start(ident[:], ID_ap[:])
        ones_row = const_pool.tile([1, 128], F32, tag="ones_row")
        nc.vector.memset(ones_row[:], 1.0)
        ones_col = nc.const_aps.tensor(1.0, (128, 1))
        wrow = const_pool.tile([1, 3 * H], F32, tag="wrow")
        nc.sync.dma_start(wrow[:], W_ap.rearrange("(a c) -> a c", a=1))
        wsb = const_pool.tile([128, 12], F32, tag="wsb")
        wsbB = const_pool.tile([128, 12], BF16, tag="wsbB")
        pwc = p_ty.tile([128, 12], F32, tag="tiny", bufs=2)
        for c in range(12):
            # [1,128] -> [128,1] transpose: K=1, so the "identity" is [1,1]
            nc.tensor.transpose(pwc[:, c:c + 1], wrow[:, ts(c, 128)],
                                ones_row[:, 0:1])
        nc.vector.tensor_copy(wsb[:], pwc[:])
        nc.vector.tensor_copy(wsbB[:], pwc[:])
        w1c = [wsb[:, k:k + 1] for k in range(KH)]
        w1cB = [wsbB[:, k:k + 1] for k in range(KH)]
        w3c = [wsb[:, 8 + k:9 + k] for k in range(KH)]
        identB = const_pool.tile([128, 128], BF16, tag="identB")
        nc.vector.tensor_copy(identB[:], ident[:])
        onesb = const_pool.tile([1, 128], BF16, tag="onesb")
        nc.vector.memset(onesb[:], 1.0)
        ones_col_b = const_pool.tile([128, 1], BF16, tag="onescolb")
        nc.vector.memset(ones_col_b[:], 1.0)
        # broadcast w2 across partitions via K=1 matmul
        W2b = const_pool.tile([128, H], F32, tag="w2b")
        pw = p_mm.tile([128, 512], F32, tag="mm", bufs=4)
        nc.tensor.matmul(pw[:], ones_row[:], wrow[:, H:2 * H],
                         start=True, stop=True)
        nc.vector.tensor_copy(W2b[:], pw[:])

        consts = (ident, identB, ones_row, onesb, ones_col,
                  ones_col_b, w3c, w1c, w1cB, W2b)
        pools = (c_pool, ct_pool, q_pool, pt_pool, sm_pool, scr_pool, a_pool,
                 ot_pool, p_mm, p_sm, p_ty)
        aps = (C_ap, Q_ap, M_ap, O_ap, consts)

        exs = [(rep, b) for rep in range(n_rep)
               for b in range(B_PER_CORE)]
        qstate = {}
        loaded = {}
        PF = 2  # examples of C prefetch ahead of compute
        for i in range(min(PF, len(exs))):
            loaded[i] = _emit_load_ex(nc, pools, aps, i, *exs[i], qstate)
        prev = None
        for i, (rep, b) in enumerate(exs):
            Qall, QallB, mall = qstate[rep]
            st = _emit_prep(nc, pools, aps, b, loaded.pop(i), Qall, QallB,
                            mall)
            j = i + PF
            if j < len(exs):
                loaded[j] = _emit_load_ex(nc, pools, aps, j, *exs[j], qstate)
            if prev is not None:
                _emit_outA(nc, pools, aps, prev)
                _emit_outB(nc, pools, aps, prev)
            prev = st
        _emit_outA(nc, pools, aps, prev)
        _emit_outB(nc, pools, aps, prev)

    nc.compile()
    return nc


_NC_CACHE: dict = {}


def _get_nc(n_rep: int = 1):
    key = ("nc", n_rep)
    if key not in _NC_CACHE:
        _NC_CACHE[key] = build_nc(n_rep)
    return _NC_CACHE[key]


def make_in_maps(C, Q, q_mask, w):
    ident = np.eye(128, dtype=np.float32)
    w = np.ascontiguousarray(w, dtype=np.float32)
    in_maps = []
    for c in range(N_CORES):
        sl = slice(c * B_PER_CORE, (c + 1) * B_PER_CORE)
        in_maps.append({
            "C": np.ascontiguousarray(C[sl], dtype=np.float32),
            "Q": np.ascontiguousarray(Q[sl], dtype=np.float32),
            "q_mask": np.ascontiguousarray(q_mask[sl], dtype=np.int32),
            "w": w,
            "ident": ident,
        })
    return in_maps


def kernel(C, Q, q_mask, w):
    nc = _get_nc(1)
    in_maps = make_in_maps(C, Q, q_mask, w)
    res = run_bass_kernel_spmd(nc, in_maps, list(range(N_CORES)))
    out = np.concatenate([res.results[c]["out"] for c in range(N_CORES)],
                         axis=0)
    return out


# revision 20
# speedup vs baseline: 1.5044x; 1.0795x over previous
"""Trainium2 Bass kernel for ContextQuestionAttention (BiDAF-style).

Reference computation (per example):
    w1, w2, w3 = w[:H], w[H:2H], w[2H:]
    S[i,j] = C[i]·w1 + Q[j]·w2 + sum_h C[i,h] Q[j,h] w3[h]
    S = where(q_mask==0, -1e9, S)
    A = softmax_j(S) @ Q
    B_att = softmax_i(max_j S); B_vec = B_att @ C
    out = concat([C, A, C*A, C*B_vec], -1)

Sharding: data-parallel over batch, 4 examples per core on 8 cores.

The kernel is HBM-bound (9.4 MB in + 33.6 MB out per core @ ~360 GB/s =>
~117 us floor), so the layout aims at keeping the DMA queues saturated:

  - ALL input loads are issued up front (constants, then C of ex0, Q+mask,
    then C of ex1..3).  That fills the DMA pipe for the first ~26 us while
    compute ramps, and removes inter-example load/store serialization.
  - out[:, 0:H] = C verbatim: streamed SBUF->HBM from the loaded C tile via
    the ACT HWDGE ring (scalar.dma_start) so it neither blocks the SP ring
    nor costs an engine copy.  C*B_vec tiles go out on the Pool SWDGE ring.
    Per-i-tile [A|C*A] bursts go on the SP ring.  Three independent DMA
    issue streams -> no head-of-line blocking.
  - V^T[j,i] = s_cq^T + s_q[j] + maskbias[j] with j on partitions:
    matmul(lhsT=(w3*Q)^T chunks, rhs=C^T chunks) in PSUM; the per-partition
    (s_q + mask) bias is folded into the Exp activation that produces
    P'T = exp(V^T).  The s_c[i] term cancels in softmax_j and is left out.
  - Big matmuls run as float32r (1 cycle/row for free-dim >= 256 vs 4 for
    fp32) via AP bitcast - 4x PE time; tolerance dwarfs tf32-ish rounding.
  - A[i,:] = P'T[:,i].T @ Q / Z'[i] - P'T is already the lhsT the A-matmul
    needs; Z' via ones-column matmuls interleaved with the A matmuls.
    C*A is fused on DVE as (pa * 1/Z') * C straight out of PSUM
    (scalar_tensor_tensor), in parallel with ACT producing A itself.
  - The E/B path (rowmax transposes, s_c, B_att, B_vec) is deferred until
    after the A-phase so first outputs leave as early as possible:
    E = exp(maxS) = rowmax_j(P') * exp(s_c); B_vec^T chunks via N=1
    matmuls contracting i; broadcast via K=1 matmul; C*B_vec on GpSimd.
  - exp() without max subtraction is safe: |S| <~ 13 for these magnitudes.
"""

import os
import sys
from contextlib import ExitStack

import numpy as np

for _p in ("/opt/trn_rl_repo", "/root/.axon_site/_ro/trn_rl_repo"):
    if os.path.isdir(_p) and _p not in sys.path:
        sys.path.append(_p)

import concourse.bass as bass
import concourse.tile as tile
from concourse import bacc, mybir
from concourse.bass_utils import run_bass_kernel_spmd

F32 = mybir.dt.float32
F32R = mybir.dt.float32r
BF16 = mybir.dt.bfloat16
I32 = mybir.dt.int32
AX = mybir.AxisListType
ALU = mybir.AluOpType
ACTF = mybir.ActivationFunctionType
ts = bass.ts

N_CORES = 8
B_TOTAL = 32
B_PER_CORE = B_TOTAL // N_CORES  # 4
CLEN = 1024
QLEN = 128
H = 512
NT = CLEN // 128  # 8 i-tiles per example
KH = H // 128     # 4 h-chunks
NEG = -1.0e9


def _r(ap):
    """Reinterpret an fp32 AP as float32r for full-rate PE matmuls."""
    return ap.bitcast(F32R)


def _emit_load_ex(nc, pools, aps, i, rep, b, qstate):
    """Input DMAs for global example index i (prefetched ~2 ahead).
    Per-rep Q/mask batch loads ride with example b==0."""
    (c_pool, ct_pool, q_pool, pt_pool, sm_pool, scr_pool, a_pool, ot_pool,
     p_mm, p_sm, p_ty) = pools
    C_ap, Q_ap, M_ap, O_ap, consts = aps

    if b == 0:
        Qall = q_pool.tile([128, B_PER_CORE * H], F32, tag="qall", bufs=2)
        QallB = q_pool.tile([128, B_PER_CORE * H], BF16, tag="qallb", bufs=2)
        mall = sm_pool.tile([128, B_PER_CORE], I32, tag="mall", bufs=2)
        nc.sync.dma_start(
            Qall[:].rearrange("p (b h) -> p b h", h=H),
            Q_ap.rearrange("b p h -> p b h"))
        nc.sync.dma_start(mall[:], M_ap.rearrange("b p -> p b"))
        nc.vector.tensor_copy(QallB[:], Qall[:])
        qstate[rep] = (Qall, QallB, mall)
    call = c_pool.tile([128, NT * H], F32, tag="call", bufs=4,
                       name=f"call_{i}")
    chalf = C_ap[b].rearrange("(g t p) h -> g p t h", g=2, p=128)
    for g in range(2):
        nc.sync.dma_start(call[:, ts(g, NT * H // 2)], chalf[g])
    return call


def _emit_prep(nc, pools, aps, b, call, Qall, QallB, mall):
    """Loads passthrough + everything up to P'T and Z'-free A inputs."""
    (c_pool, ct_pool, q_pool, pt_pool, sm_pool, scr_pool, a_pool, ot_pool,
     p_mm, p_sm, p_ty) = pools
    C_ap, Q_ap, M_ap, O_ap, consts = aps
    (ident, identB, ones_row, onesb, ones_col, ones_col_b, w3c,
     w1c, w1cB, W2b) = consts

    Csb = [call[:, ts(t, H)] for t in range(NT)]
    Qsb = Qall[:, ts(b, H)]
    QsbB = QallB[:, ts(b, H)]

    # ---- stream C straight back out to out[:, 0:H] ----
    for g in range(2):
        nc.sync.dma_start(
            O_ap[b][ts(g, CLEN // 2), 0:H].rearrange("(t p) h -> p t h",
                                                     p=128),
            call[:, ts(g, NT * H // 2)].rearrange("p (t h) -> p t h", h=H))

    # ---- mask bias + s_q (per-partition over j) ----
    mskf = sm_pool.tile([128, 1], F32, tag="mskf", bufs=2)
    nc.vector.tensor_copy(mskf[:], mall[:, b:b + 1])
    mb = sm_pool.tile([128, 1], F32, tag="mb", bufs=2)
    # (mask - 1) * 1e9  -> 0 where mask==1, -1e9 where mask==0
    nc.vector.tensor_scalar(
        out=mb[:], in0=mskf[:], scalar1=1.0, scalar2=1.0e9,
        op0=ALU.subtract, op1=ALU.mult)
    scr = scr_pool.tile([128, H], F32, tag="scr", bufs=2)
    sq = sm_pool.tile([128, 1], F32, tag="sq", bufs=2)
    sqe = sm_pool.tile([128, 1], F32, tag="sqe", bufs=2)
    # sqe[j] = mb[j] + sum_h Q[j,h] * w2[h]
    nc.vector.tensor_mul(scr[:], Qsb, W2b[:])
    nc.vector.reduce_sum(sq[:], scr[:], axis=AX.X)
    nc.vector.tensor_add(sqe[:], sq[:], mb[:])

    # ---- (w3 * Q)^T chunks ----
    QW3T = q_pool.tile([128, H], BF16, tag="qw3t", bufs=2)
    for k in range(KH):
        pqt = p_sm.tile([128, 128], F32, tag="sm", bufs=2)
        nc.tensor.transpose(pqt[:], Qsb[:, ts(k, 128)], ident[:])
        nc.vector.tensor_scalar_mul(QW3T[:, ts(k, 128)], pqt[:], w3c[k][:])

    # ---- C^T chunks + S^T matmul + fused bias/exp -> P'T, per half ----
    CT = [ct_pool.tile([128, CLEN], BF16, tag=f"ct{k}", bufs=2,
                       name=f"ct{k}_{b}")
          for k in range(KH)]
    PT = pt_pool.tile([128, CLEN], BF16, tag="pt", bufs=2)
    for half in range(2):
        for k in range(KH):
            pct = p_mm.tile([128, 512], F32, tag="mm", bufs=4)
            for tt in range(4):
                t = half * 4 + tt
                nc.tensor.transpose(
                    pct[:, ts(tt, 128)], Csb[t][:, ts(k, 128)], ident[:])
            if k % 2 == 0:
                nc.scalar.copy(CT[k][:, ts(half, 512)], pct[:])
            else:
                nc.vector.tensor_copy(CT[k][:, ts(half, 512)], pct[:])
        pst = p_mm.tile([128, 512], F32, tag="mm", bufs=4)
        for k in range(KH):
            nc.tensor.matmul(
                pst[:], QW3T[:, ts(k, 128)], CT[k][:, ts(half, 512)],
                start=(k == 0), stop=(k == KH - 1))
        # P'T = exp(s_cq^T + s_q + maskbias)
        nc.scalar.activation(PT[:, ts(half, 512)], pst[:], ACTF.Exp,
                             bias=sqe[:], scale=1.0)

    return dict(b=b, call=call, Csb=Csb, Qsb=Qsb, QsbB=QsbB,
                CT=CT, PT=PT)


def _emit_outA(nc, pools, aps, st):
    (c_pool, ct_pool, q_pool, pt_pool, sm_pool, scr_pool, a_pool, ot_pool,
     p_mm, p_sm, p_ty) = pools
    C_ap, Q_ap, M_ap, O_ap, consts = aps
    (ident, identB, ones_row, onesb, ones_col, ones_col_b, w3c,
     w1c, w1cB, W2b) = consts
    b, Csb, QsbB, PT = st["b"], st["Csb"], st["QsbB"], st["PT"]

    # ---- A path per i-tile; stage [A|C*A] and DMA as one 4KB-row burst ----
    # Z' for 4 tiles batched per PSUM bank, interleaved with the A matmuls.
    RZP = sm_pool.tile([128, NT], F32, tag="rzp", bufs=2)
    for g in range(2):
        pzg = p_ty.tile([128, 4], F32, tag="tiny", bufs=2)
        for tt in range(4):
            t = g * 4 + tt
            nc.tensor.matmul(pzg[:, tt:tt + 1], PT[:, ts(t, 128)],
                             ones_col_b[:], start=True, stop=True)
        nc.vector.reciprocal(RZP[:, ts(g, 4)], pzg[:])
        for tt in range(4):
            t = g * 4 + tt
            pa = p_mm.tile([128, 512], F32, tag="mm", bufs=4)
            nc.tensor.matmul(pa[:], PT[:, ts(t, 128)], QsbB,
                             start=True, stop=True)
            ot = ot_pool.tile([128, 2 * H], F32, tag="ot", bufs=6)
            # A = pa / Z' on ACT; C*A = (pa / Z') * C fused on DVE from PSUM
            nc.scalar.mul(ot[:, 0:H], pa[:], RZP[:, t:t + 1])
            nc.vector.scalar_tensor_tensor(
                out=ot[:, H:2 * H], in0=pa[:], scalar=RZP[:, t:t + 1],
                in1=Csb[t][:], op0=ALU.mult, op1=ALU.mult)
            nc.sync.dma_start(O_ap[b, ts(t, 128), H:3 * H], ot[:])


def _emit_outB(nc, pools, aps, st):
    (c_pool, ct_pool, q_pool, pt_pool, sm_pool, scr_pool, a_pool, ot_pool,
     p_mm, p_sm, p_ty) = pools
    C_ap, Q_ap, M_ap, O_ap, consts = aps
    (ident, identB, ones_row, onesb, ones_col, ones_col_b, w3c,
     w1c, w1cB, W2b) = consts
    b, Csb, CT, PT = st["b"], st["Csb"], st["CT"], st["PT"]

    # ---- s_c columns on PE (reuses CT): SC[:, t] = C_t @ w1 ----
    # 8 sequential accumulation groups share one PSUM bank (one per column)
    SC = sm_pool.tile([128, NT], F32, tag="sc", bufs=2)
    psc8 = p_ty.tile([128, NT], F32, tag="tiny", bufs=2)
    for t in range(NT):
        for k in range(KH):
            nc.tensor.matmul(psc8[:, t:t + 1], CT[k][:, ts(t, 128)],
                             w1cB[k][:], start=(k == 0), stop=(k == KH - 1))
    nc.scalar.copy(SC[:], psc8[:])

    # ---- row max of P' natural (PE transposes, batched 4-per-psum-bank,
    # one segmented reduce per batch) ----
    MXE = sm_pool.tile([128, NT], F32, tag="mxe", bufs=2)
    for g in range(2):
        ppn = p_sm.tile([128, 512], BF16, tag="sm", bufs=2)
        for tt in range(4):
            t = g * 4 + tt
            nc.tensor.transpose(ppn[:, ts(tt, 128)], PT[:, ts(t, 128)],
                                identB[:])
        nc.vector.reduce_max(
            MXE[:, ts(g, 4)], ppn[:].rearrange("p (t x) -> p t x", x=128),
            axis=AX.X)

    # ---- E = exp(maxS) = rowmax(P') * exp(s_c) ----
    esc = sm_pool.tile([128, NT], F32, tag="esc", bufs=2)
    nc.scalar.activation(esc[:], SC[:], ACTF.Exp)
    E = sm_pool.tile([128, NT], F32, tag="e", bufs=2)
    nc.vector.tensor_mul(E[:], MXE[:], esc[:])

    # ---- B path: B_vec^T chunks via N=1 matmuls contracting i ----
    # 4 sequential accumulation groups (one per chunk column) in one bank
    pbt4 = p_ty.tile([128, KH], F32, tag="tiny", bufs=2)
    for k in range(KH):
        for t in range(NT):
            nc.tensor.matmul(pbt4[:, k:k + 1], Csb[t][:, ts(k, 128)],
                             E[:, t:t + 1], start=(t == 0),
                             stop=(t == NT - 1))
    btc = sm_pool.tile([128, KH], F32, tag="btc", bufs=2)
    nc.scalar.copy(btc[:], pbt4[:])
    Btrow = sm_pool.tile([1, H], BF16, tag="btrow", bufs=2)
    for k in range(KH):
        ptr = p_sm.tile([1, 128], F32, tag="sm", bufs=2)
        nc.tensor.transpose(ptr[:], btc[:, k:k + 1], ident[:])
        nc.scalar.copy(Btrow[:, ts(k, 128)], ptr[:])
    # Z2 = sum(E): free-dim reduce on DVE, partition reduce via one matmul
    rse = sm_pool.tile([128, 1], F32, tag="rse", bufs=2)
    nc.vector.reduce_sum(rse[:], E[:], axis=AX.X)
    pz2 = p_ty.tile([1, 1], F32, tag="tiny", bufs=2)
    nc.tensor.matmul(pz2[:], rse[:], ones_col, start=True, stop=True)
    z2sb = sm_pool.tile([1, 1], BF16, tag="z2", bufs=2)
    nc.scalar.copy(z2sb[:], pz2[:])
    # broadcast row -> all partitions with K=1 matmuls
    pbb = p_mm.tile([128, 512], F32, tag="mm", bufs=4)
    nc.tensor.matmul(pbb[:], onesb[:], Btrow[:], start=True, stop=True)
    pzb = p_ty.tile([128, 1], F32, tag="tiny", bufs=2)
    nc.tensor.matmul(pzb[:], onesb[:], z2sb[:], start=True, stop=True)
    rzb = sm_pool.tile([128, 1], F32, tag="rzb", bufs=2)
    nc.vector.reciprocal(rzb[:], pzb[:])
    Bb = a_pool.tile([128, H], F32, tag="bb", bufs=2)
    nc.scalar.mul(Bb[:], pbb[:], rzb[:])
    # C*B_vec tiles split GpSimd/DVE (safe: pipelined emission keeps the
    # next example's prep ahead of these in each engine stream), one DMA
    cb = a_pool.tile([128, NT * H], F32, tag="cb", bufs=2)
    for t in range(NT):
        eng = nc.gpsimd if t % 2 == 0 else nc.vector
        eng.tensor_mul(cb[:, ts(t, H)], Csb[t][:], Bb[:])
    nc.sync.dma_start(
        O_ap[b][:, 3 * H:4 * H].rearrange("(t p) h -> p t h", p=128),
        cb[:].rearrange("p (t h) -> p t h", h=H))


def build_nc(n_rep: int = 1):
    nc = bacc.Bacc("TRN2", target_bir_lowering=False, debug=False,
                   num_devices=N_CORES)
    C_ap = nc.dram_tensor("C", [B_PER_CORE, CLEN, H], F32,
                          kind="ExternalInput").ap()
    Q_ap = nc.dram_tensor("Q", [B_PER_CORE, QLEN, H], F32,
                          kind="ExternalInput").ap()
    M_ap = nc.dram_tensor("q_mask", [B_PER_CORE, QLEN], I32,
                          kind="ExternalInput").ap()
    W_ap = nc.dram_tensor("w", [3 * H], F32, kind="ExternalInput").ap()
    ID_ap = nc.dram_tensor("ident", [128, 128], F32,
                           kind="ExternalInput").ap()
    O_ap = nc.dram_tensor("out", [B_PER_CORE, CLEN, 4 * H], F32,
                          kind="ExternalOutput").ap()

    with tile.TileContext(nc) as tc, ExitStack() as ctx:
        const_pool = ctx.enter_context(tc.tile_pool(name="const", bufs=1))
        c_pool = ctx.enter_context(tc.tile_pool(name="cpool",
                                                bufs=B_PER_CORE))
        ct_pool = ctx.enter_context(tc.tile_pool(name="ctpool", bufs=2))
        q_pool = ctx.enter_context(tc.tile_pool(name="qpool", bufs=2))
        pt_pool = ctx.enter_context(tc.tile_pool(name="ptpool", bufs=2))
        sm_pool = ctx.enter_context(tc.tile_pool(name="smpool", bufs=2))
        scr_pool = ctx.enter_context(tc.tile_pool(name="scrpool", bufs=2))
        a_pool = ctx.enter_context(tc.tile_pool(name="apool", bufs=3))
        ot_pool = ctx.enter_context(tc.tile_pool(name="otpool", bufs=3))
        p_mm = ctx.enter_context(tc.tile_pool(name="pmm", bufs=4,
                                              space="PSUM"))
        p_sm = ctx.enter_context(tc.tile_pool(name="psm", bufs=2,
                                              space="PSUM"))
        p_ty = ctx.enter_context(tc.tile_pool(name="pty", bufs=2,
                                              space="PSUM"))

        # constants: ident + w as a single 6KB row (1 descriptor), then
        # w1/w3 columns via PE transposes of the row chunks
        ident = const_pool.tile([128, 128], F32, tag="ident")
        nc.sync.dma_start(ident[:], ID_ap[:])
        ones_row = const_pool.tile([1, 128], F32, tag="ones_row")
        nc.vector.memset(ones_row[:], 1.0)
        ones_col = nc.const_aps.tensor(1.0, (128, 1))
        wrow = const_pool.tile([1, 3 * H], F32, tag="wrow")
        nc.sync.dma_start(wrow[:], W_ap.rearrange("(a c) -> a c", a=1))
        wsb = const_pool.tile([128, 12], F32, tag="wsb")
        wsbB = const_pool.tile([128, 12], BF16, tag="wsbB")
        pwc = p_ty.tile([128, 12], F32, tag="tiny", bufs=2)
        for c in range(12):
            # [1,128] -> [128,1] transpose: K=1, so the "identity" is [1,1]
            nc.tensor.transpose(pwc[:, c:c + 1], wrow[:, ts(c, 128)],
                                ones_row[:, 0:1])
        nc.vector.tensor_copy(wsb[:], pwc[:])
        nc.vector.tensor_copy(wsbB[:], pwc[:])
        w1c = [wsb[:, k:k + 1] for k in range(KH)]
        w1cB = [wsbB[:, k:k + 1] for k in range(KH)]
        w3c = [wsb[:, 8 + k:9 + k] for k in range(KH)]
        identB = const_pool.tile([128, 128], BF16, tag="identB")
        nc.vector.tensor_copy(identB[:], ident[:])
        onesb = const_pool.tile([1, 128], BF16, tag="onesb")
        nc.vector.memset(onesb[:], 1.0)
        ones_col_b = const_pool.tile([128, 1], BF16, tag="onescolb")
        nc.vector.memset(ones_col_b[:], 1.0)
        # broadcast w2 across partitions via K=1 matmul
        W2b = const_pool.tile([128, H], F32, tag="w2b")
        pw = p_mm.tile([128, 512], F32, tag="mm", bufs=4)
        nc.tensor.matmul(pw[:], ones_row[:], wrow[:, H:2 * H],
                         start=True, stop=True)
        nc.vector.tensor_copy(W2b[:], pw[:])

        consts = (ident, identB, ones_row, onesb, ones_col,
                  ones_col_b, w3c, w1c, w1cB, W2b)
        pools = (c_pool, ct_pool, q_pool, pt_pool, sm_pool, scr_pool, a_pool,
                 ot_pool, p_mm, p_sm, p_ty)
        aps = (C_ap, Q_ap, M_ap, O_ap, consts)

        exs = [(rep, b) for rep in range(n_rep)
               for b in range(B_PER_CORE)]
        qstate = {}
        loaded = {}
        PF = 2  # examples of C prefetch ahead of compute
        for i in range(min(PF, len(exs))):
            loaded[i] = _emit_load_ex(nc, pools, aps, i, *exs[i], qstate)
        prev = None
        for i, (rep, b) in enumerate(exs):
            Qall, QallB, mall = qstate[rep]
            st = _emit_prep(nc, pools, aps, b, loaded.pop(i), Qall, QallB,
                            mall)
            j = i + PF
            if j < len(exs):
                loaded[j] = _emit_load_ex(nc, pools, aps, j, *exs[j], qstate)
            if prev is not None:
                _emit_outA(nc, pools, aps, prev)
                _emit_outB(nc, pools, aps, prev)
            prev = st
        _emit_outA(nc, pools, aps, prev)
        _emit_outB(nc, pools, aps, prev)

    nc.compile()
    return nc


_NC_CACHE: dict = {}


def _get_nc(n_rep: int = 1):
    key = ("nc", n_rep)
    if key not in _NC_CACHE:
        _NC_CACHE[key] = build_nc(n_rep)
    return _NC_CACHE[key]


def make_in_maps(C, Q, q_mask, w):
    ident = np.eye(128, dtype=np.float32)
    w = np.ascontiguousarray(w, dtype=np.float32)
    in_maps = []
    for c in range(N_CORES):
        sl = slice(c * B_PER_CORE, (c + 1) * B_PER_CORE)
        in_maps.append({
            "C": np.ascontiguousarray(C[sl], dtype=np.float32),
            "Q": np.ascontiguousarray(Q[sl], dtype=np.float32),
            "q_mask": np.ascontiguousarray(q_mask[sl], dtype=np.int32),
            "w": w,
            "ident": ident,
        })
    return in_maps


def kernel(C, Q, q_mask, w):
    nc = _get_nc(1)
    in_maps = make_in_maps(C, Q, q_mask, w)
    res = run_bass_kernel_spmd(nc, in_maps, list(range(N_CORES)))
    out = np.concatenate([res.results[c]["out"] for c in range(N_CORES)],
                         axis=0)
    return out


# revision 26
# speedup vs baseline: 2.5896x; 1.7214x over previous
"""Trainium2 Bass kernel for ContextQuestionAttention (BiDAF-style).

Reference computation (per example):
    w1, w2, w3 = w[:H], w[H:2H], w[2H:]
    S[i,j] = C[i]·w1 + Q[j]·w2 + sum_h C[i,h] Q[j,h] w3[h]
    S = where(q_mask==0, -1e9, S)
    A = softmax_j(S) @ Q
    B_att = softmax_i(max_j S); B_vec = B_att @ C
    out = concat([C, A, C*A, C*B_vec], -1)

Sharding: data-parallel over batch, 4 examples per core on 8 cores.

The kernel is HBM-bound (9.4 MB in + 33.6 MB out per core @ ~360 GB/s =>
~117 us floor), so the layout aims at keeping the DMA queues saturated:

  - ALL input loads are issued up front (constants, then C of ex0, Q+mask,
    then C of ex1..3).  That fills the DMA pipe for the first ~26 us while
    compute ramps, and removes inter-example load/store serialization.
  - out[:, 0:H] = C verbatim: streamed SBUF->HBM from the loaded C tile via
    the ACT HWDGE ring (scalar.dma_start) so it neither blocks the SP ring
    nor costs an engine copy.  C*B_vec tiles go out on the Pool SWDGE ring.
    Per-i-tile [A|C*A] bursts go on the SP ring.  Three independent DMA
    issue streams -> no head-of-line blocking.
  - V^T[j,i] = s_cq^T + s_q[j] + maskbias[j] with j on partitions:
    matmul(lhsT=(w3*Q)^T chunks, rhs=C^T chunks) in PSUM; the per-partition
    (s_q + mask) bias is folded into the Exp activation that produces
    P'T = exp(V^T).  The s_c[i] term cancels in softmax_j and is left out.
  - Big matmuls run as float32r (1 cycle/row for free-dim >= 256 vs 4 for
    fp32) via AP bitcast - 4x PE time; tolerance dwarfs tf32-ish rounding.
  - A[i,:] = P'T[:,i].T @ Q / Z'[i] - P'T is already the lhsT the A-matmul
    needs; Z' via ones-column matmuls interleaved with the A matmuls.
    C*A is fused on DVE as (pa * 1/Z') * C straight out of PSUM
    (scalar_tensor_tensor), in parallel with ACT producing A itself.
  - The E/B path (rowmax transposes, s_c, B_att, B_vec) is deferred until
    after the A-phase so first outputs leave as early as possible:
    E = exp(maxS) = rowmax_j(P') * exp(s_c); B_vec^T chunks via N=1
    matmuls contracting i; broadcast via K=1 matmul; C*B_vec on GpSimd.
  - exp() without max subtraction is safe: |S| <~ 13 for these magnitudes.
"""

import os
import sys
from contextlib import ExitStack

import numpy as np

for _p in ("/opt/trn_rl_repo", "/root/.axon_site/_ro/trn_rl_repo"):
    if os.path.isdir(_p) and _p not in sys.path:
        sys.path.append(_p)

import concourse.bass as bass
import concourse.tile as tile
from concourse import bacc, mybir
from concourse.bass_utils import run_bass_kernel_spmd

F32 = mybir.dt.float32
F32R = mybir.dt.float32r
BF16 = mybir.dt.bfloat16
I32 = mybir.dt.int32
AX = mybir.AxisListType
ALU = mybir.AluOpType
ACTF = mybir.ActivationFunctionType
ts = bass.ts

N_CORES = 8
B_TOTAL = 32
B_PER_CORE = B_TOTAL // N_CORES  # 4
CLEN = 1024
QLEN = 128
H = 512
NT = CLEN // 128  # 8 i-tiles per example
KH = H // 128     # 4 h-chunks
NEG = -1.0e9


def _r(ap):
    """Reinterpret an fp32 AP as float32r for full-rate PE matmuls."""
    return ap.bitcast(F32R)


def _emit_load_ex(nc, pools, aps, i, rep, b, qstate):
    """Input DMAs for global example index i (prefetched ~2 ahead).
    Per-rep Q/mask batch loads ride with example b==0."""
    (c_pool, ct_pool, q_pool, pt_pool, sm_pool, scr_pool, a_pool, ot_pool,
     p_mm, p_sm, p_ty) = pools
    C_ap, Q_ap, M_ap, O_ap, consts = aps

    if b == 0:
        Qall = q_pool.tile([128, B_PER_CORE * H], F32, tag="qall", bufs=2)
        QallB = q_pool.tile([128, B_PER_CORE * H], BF16, tag="qallb", bufs=2)
        mall = sm_pool.tile([128, B_PER_CORE], I32, tag="mall", bufs=2)
        nc.sync.dma_start(
            Qall[:].rearrange("p (b h) -> p b h", h=H),
            Q_ap.rearrange("b p h -> p b h"))
        nc.sync.dma_start(mall[:], M_ap.rearrange("b p -> p b"))
        nc.vector.tensor_copy(QallB[:], Qall[:])
        qstate[rep] = (Qall, QallB, mall)
    call = c_pool.tile([128, NT * H], F32, tag="call", bufs=4,
                       name=f"call_{i}")
    nc.sync.dma_start(call[:].rearrange("p (t h) -> p t h", h=H),
                      C_ap[b].rearrange("(t p) h -> p t h", p=128))
    return call


def _emit_prep(nc, pools, aps, b, call, Qall, QallB, mall):
    """Loads passthrough + everything up to P'T and Z'-free A inputs."""
    (c_pool, ct_pool, q_pool, pt_pool, sm_pool, scr_pool, a_pool, ot_pool,
     p_mm, p_sm, p_ty) = pools
    C_ap, Q_ap, M_ap, O_ap, consts = aps
    (ident, identB, ones_row, onesb, ones_col, ones_col_b, w3c,
     w1c, w1cB, W2b) = consts

    Csb = [call[:, ts(t, H)] for t in range(NT)]
    Qsb = Qall[:, ts(b, H)]
    QsbB = QallB[:, ts(b, H)]

    # ---- stream C straight back out to out[:, 0:H] ----
    nc.sync.dma_start(
        O_ap[b][:, 0:H].rearrange("(t p) h -> p t h", p=128),
        call[:].rearrange("p (t h) -> p t h", h=H))

    # ---- mask bias + s_q (per-partition over j) ----
    mskf = sm_pool.tile([128, 1], F32, tag="mskf", bufs=2)
    nc.vector.tensor_copy(mskf[:], mall[:, b:b + 1])
    mb = sm_pool.tile([128, 1], F32, tag="mb", bufs=2)
    # (mask - 1) * 1e9  -> 0 where mask==1, -1e9 where mask==0
    nc.vector.tensor_scalar(
        out=mb[:], in0=mskf[:], scalar1=1.0, scalar2=1.0e9,
        op0=ALU.subtract, op1=ALU.mult)
    scr = scr_pool.tile([128, H], F32, tag="scr", bufs=2)
    sq = sm_pool.tile([128, 1], F32, tag="sq", bufs=2)
    sqe = sm_pool.tile([128, 1], F32, tag="sqe", bufs=2)
    # sqe[j] = mb[j] + sum_h Q[j,h] * w2[h]
    nc.vector.tensor_mul(scr[:], Qsb, W2b[:])
    nc.vector.reduce_sum(sq[:], scr[:], axis=AX.X)
    nc.vector.tensor_add(sqe[:], sq[:], mb[:])

    # ---- (w3 * Q)^T chunks ----
    QW3T = q_pool.tile([128, H], BF16, tag="qw3t", bufs=2)
    for k in range(KH):
        pqt = p_sm.tile([128, 128], F32, tag="sm", bufs=2)
        nc.tensor.transpose(pqt[:], Qsb[:, ts(k, 128)], ident[:])
        nc.vector.tensor_scalar_mul(QW3T[:, ts(k, 128)], pqt[:], w3c[k][:])

    # ---- C^T chunks + S^T matmul + fused bias/exp -> P'T, per half ----
    CT = [ct_pool.tile([128, CLEN], BF16, tag=f"ct{k}", bufs=2,
                       name=f"ct{k}_{b}")
          for k in range(KH)]
    PT = pt_pool.tile([128, CLEN], BF16, tag="pt", bufs=2)
    for half in range(2):
        for k in range(KH):
            pct = p_mm.tile([128, 512], F32, tag="mm", bufs=4)
            for tt in range(4):
                t = half * 4 + tt
                nc.tensor.transpose(
                    pct[:, ts(tt, 128)], Csb[t][:, ts(k, 128)], ident[:])
            if k % 2 == 0:
                nc.scalar.copy(CT[k][:, ts(half, 512)], pct[:])
            else:
                nc.vector.tensor_copy(CT[k][:, ts(half, 512)], pct[:])
        pst = p_mm.tile([128, 512], F32, tag="mm", bufs=4)
        for k in range(KH):
            nc.tensor.matmul(
                pst[:], QW3T[:, ts(k, 128)], CT[k][:, ts(half, 512)],
                start=(k == 0), stop=(k == KH - 1))
        # P'T = exp(s_cq^T + s_q + maskbias)
        nc.scalar.activation(PT[:, ts(half, 512)], pst[:], ACTF.Exp,
                             bias=sqe[:], scale=1.0)

    return dict(b=b, call=call, Csb=Csb, Qsb=Qsb, QsbB=QsbB,
                CT=CT, PT=PT)


def _emit_outA(nc, pools, aps, st):
    (c_pool, ct_pool, q_pool, pt_pool, sm_pool, scr_pool, a_pool, ot_pool,
     p_mm, p_sm, p_ty) = pools
    C_ap, Q_ap, M_ap, O_ap, consts = aps
    (ident, identB, ones_row, onesb, ones_col, ones_col_b, w3c,
     w1c, w1cB, W2b) = consts
    b, Csb, QsbB, PT = st["b"], st["Csb"], st["QsbB"], st["PT"]

    # ---- A path per i-tile; stage [A|C*A] and DMA as one 4KB-row burst ----
    # Z' for 4 tiles batched per PSUM bank, interleaved with the A matmuls.
    RZP = sm_pool.tile([128, NT], F32, tag="rzp", bufs=2)
    for g in range(2):
        pzg = p_ty.tile([128, 4], F32, tag="tiny", bufs=2)
        for tt in range(4):
            t = g * 4 + tt
            nc.tensor.matmul(pzg[:, tt:tt + 1], PT[:, ts(t, 128)],
                             ones_col_b[:], start=True, stop=True)
        nc.vector.reciprocal(RZP[:, ts(g, 4)], pzg[:])
        for pair in range(2):
            ot = ot_pool.tile([128, 4 * H], F32, tag="ot", bufs=3)
            for u in range(2):
                t = g * 4 + pair * 2 + u
                pa = p_mm.tile([128, 512], F32, tag="mm", bufs=4)
                nc.tensor.matmul(pa[:], PT[:, ts(t, 128)], QsbB,
                                 start=True, stop=True)
                # A = pa/Z' on ACT; C*A = (pa/Z')*C fused on DVE from PSUM
                nc.scalar.mul(ot[:, ts(2 * u, H)], pa[:], RZP[:, t:t + 1])
                nc.vector.scalar_tensor_tensor(
                    out=ot[:, ts(2 * u + 1, H)], in0=pa[:],
                    scalar=RZP[:, t:t + 1], in1=Csb[t][:],
                    op0=ALU.mult, op1=ALU.mult)
            gp = g * 2 + pair
            nc.sync.dma_start(
                O_ap[b][ts(gp, 2 * 128), H:3 * H].rearrange(
                    "(u p) h -> p u h", p=128),
                ot[:].rearrange("p (u h) -> p u h", h=2 * H))


def _emit_outB(nc, pools, aps, st):
    (c_pool, ct_pool, q_pool, pt_pool, sm_pool, scr_pool, a_pool, ot_pool,
     p_mm, p_sm, p_ty) = pools
    C_ap, Q_ap, M_ap, O_ap, consts = aps
    (ident, identB, ones_row, onesb, ones_col, ones_col_b, w3c,
     w1c, w1cB, W2b) = consts
    b, Csb, CT, PT = st["b"], st["Csb"], st["CT"], st["PT"]

    # ---- s_c columns on PE (reuses CT): SC[:, t] = C_t @ w1 ----
    # 8 sequential accumulation groups share one PSUM bank (one per column)
    SC = sm_pool.tile([128, NT], F32, tag="sc", bufs=2)
    psc8 = p_ty.tile([128, NT], F32, tag="tiny", bufs=2)
    for t in range(NT):
        for k in range(KH):
            nc.tensor.matmul(psc8[:, t:t + 1], CT[k][:, ts(t, 128)],
                             w1cB[k][:], start=(k == 0), stop=(k == KH - 1))
    nc.scalar.copy(SC[:], psc8[:])

    # ---- row max of P' natural (PE transposes, batched 4-per-psum-bank,
    # one segmented reduce per batch) ----
    MXE = sm_pool.tile([128, NT], F32, tag="mxe", bufs=2)
    for g in range(2):
        ppn = p_sm.tile([128, 512], BF16, tag="sm", bufs=2)
        for tt in range(4):
            t = g * 4 + tt
            nc.tensor.transpose(ppn[:, ts(tt, 128)], PT[:, ts(t, 128)],
                                identB[:])
        nc.vector.reduce_max(
            MXE[:, ts(g, 4)], ppn[:].rearrange("p (t x) -> p t x", x=128),
            axis=AX.X)

    # ---- E = exp(maxS) = rowmax(P') * exp(s_c) ----
    esc = sm_pool.tile([128, NT], F32, tag="esc", bufs=2)
    nc.scalar.activation(esc[:], SC[:], ACTF.Exp)
    E = sm_pool.tile([128, NT], F32, tag="e", bufs=2)
    nc.vector.tensor_mul(E[:], MXE[:], esc[:])

    # ---- B path: B_vec^T chunks via N=1 matmuls contracting i ----
    # 4 sequential accumulation groups (one per chunk column) in one bank
    pbt4 = p_ty.tile([128, KH], F32, tag="tiny", bufs=2)
    for k in range(KH):
        for t in range(NT):
            nc.tensor.matmul(pbt4[:, k:k + 1], Csb[t][:, ts(k, 128)],
                             E[:, t:t + 1], start=(t == 0),
                             stop=(t == NT - 1))
    btc = sm_pool.tile([128, KH], F32, tag="btc", bufs=2)
    nc.scalar.copy(btc[:], pbt4[:])
    # 4 transposes into one psum row tile (4 column groups), one copy
    ptr4 = p_sm.tile([1, H], F32, tag="sm", bufs=2)
    for k in range(KH):
        nc.tensor.transpose(ptr4[:, ts(k, 128)], btc[:, k:k + 1], ident[:])
    Btrow = sm_pool.tile([1, H], BF16, tag="btrow", bufs=2)
    nc.scalar.copy(Btrow[:], ptr4[:])
    # Z2 = sum(E): free-dim reduce on DVE, partition reduce via one matmul
    rse = sm_pool.tile([128, 1], F32, tag="rse", bufs=2)
    nc.vector.reduce_sum(rse[:], E[:], axis=AX.X)
    pz2 = p_ty.tile([1, 1], F32, tag="tiny", bufs=2)
    nc.tensor.matmul(pz2[:], rse[:], ones_col, start=True, stop=True)
    z2sb = sm_pool.tile([1, 1], BF16, tag="z2", bufs=2)
    nc.scalar.copy(z2sb[:], pz2[:])
    # broadcast row -> all partitions with K=1 matmuls
    pbb = p_mm.tile([128, 512], F32, tag="mm", bufs=4)
    nc.tensor.matmul(pbb[:], onesb[:], Btrow[:], start=True, stop=True)
    pzb = p_ty.tile([128, 1], F32, tag="tiny", bufs=2)
    nc.tensor.matmul(pzb[:], onesb[:], z2sb[:], start=True, stop=True)
    rzb = sm_pool.tile([128, 1], F32, tag="rzb", bufs=2)
    nc.vector.reciprocal(rzb[:], pzb[:])
    Bb = a_pool.tile([128, H], F32, tag="bb", bufs=2)
    nc.scalar.mul(Bb[:], pbb[:], rzb[:])
    # C*B_vec tiles split GpSimd/DVE (safe: pipelined emission keeps the
    # next example's prep ahead of these in each engine stream), one DMA
    cb = a_pool.tile([128, NT * H], F32, tag="cb", bufs=2)
    for t in range(NT):
        eng = nc.gpsimd if t % 2 == 0 else nc.vector
        eng.tensor_mul(cb[:, ts(t, H)], Csb[t][:], Bb[:])
    nc.sync.dma_start(
        O_ap[b][:, 3 * H:4 * H].rearrange("(t p) h -> p t h", p=128),
        cb[:].rearrange("p (t h) -> p t h", h=H))


def build_nc(n_rep: int = 1):
    nc = bacc.Bacc("TRN2", target_bir_lowering=False, debug=False,
                   num_devices=N_CORES)
    C_ap = nc.dram_tensor("C", [B_PER_CORE, CLEN, H], F32,
                          kind="ExternalInput").ap()
    Q_ap = nc.dram_tensor("Q", [B_PER_CORE, QLEN, H], F32,
                          kind="ExternalInput").ap()
    M_ap = nc.dram_tensor("q_mask", [B_PER_CORE, QLEN], I32,
                          kind="ExternalInput").ap()
    W_ap = nc.dram_tensor("w", [3 * H], F32, kind="ExternalInput").ap()
    ID_ap = nc.dram_tensor("ident", [128, 128], F32,
                           kind="ExternalInput").ap()
    O_ap = nc.dram_tensor("out", [B_PER_CORE, CLEN, 4 * H], F32,
                          kind="ExternalOutput").ap()

    with tile.TileContext(nc) as tc, ExitStack() as ctx:
        const_pool = ctx.enter_context(tc.tile_pool(name="const", bufs=1))
        c_pool = ctx.enter_context(tc.tile_pool(name="cpool",
                                                bufs=B_PER_CORE))
        ct_pool = ctx.enter_context(tc.tile_pool(name="ctpool", bufs=2))
        q_pool = ctx.enter_context(tc.tile_pool(name="qpool", bufs=2))
        pt_pool = ctx.enter_context(tc.tile_pool(name="ptpool", bufs=2))
        sm_pool = ctx.enter_context(tc.tile_pool(name="smpool", bufs=2))
        scr_pool = ctx.enter_context(tc.tile_pool(name="scrpool", bufs=2))
        a_pool = ctx.enter_context(tc.tile_pool(name="apool", bufs=3))
        ot_pool = ctx.enter_context(tc.tile_pool(name="otpool", bufs=3))
        p_mm = ctx.enter_context(tc.tile_pool(name="pmm", bufs=4,
                                              space="PSUM"))
        p_sm = ctx.enter_context(tc.tile_pool(name="psm", bufs=2,
                                              space="PSUM"))
        p_ty = ctx.enter_context(tc.tile_pool(name="pty", bufs=2,
                                              space="PSUM"))

        # constants: ident + w as a single 6KB row (1 descriptor), then
        # w1/w3 columns via PE transposes of the row chunks
        ident = const_pool.tile([128, 128], F32, tag="ident")
        nc.sync.dma_start(ident[:], ID_ap[:])
        ones_row = const_pool.tile([1, 128], F32, tag="ones_row")
        nc.vector.memset(ones_row[:], 1.0)
        ones_col = nc.const_aps.tensor(1.0, (128, 1))
        wrow = const_pool.tile([1, 3 * H], F32, tag="wrow")
        nc.sync.dma_start(wrow[:], W_ap.rearrange("(a c) -> a c", a=1))
        wsb = const_pool.tile([128, 12], F32, tag="wsb")
        wsbB = const_pool.tile([128, 12], BF16, tag="wsbB")
        pwc = p_ty.tile([128, 12], F32, tag="tiny", bufs=2)
        for c in range(12):
            # [1,128] -> [128,1] transpose: K=1, so the "identity" is [1,1]
            nc.tensor.transpose(pwc[:, c:c + 1], wrow[:, ts(c, 128)],
                                ones_row[:, 0:1])
        nc.vector.tensor_copy(wsb[:], pwc[:])
        nc.vector.tensor_copy(wsbB[:], pwc[:])
        w1c = [wsb[:, k:k + 1] for k in range(KH)]
        w1cB = [wsbB[:, k:k + 1] for k in range(KH)]
        w3c = [wsb[:, 8 + k:9 + k] for k in range(KH)]
        identB = const_pool.tile([128, 128], BF16, tag="identB")
        nc.vector.tensor_copy(identB[:], ident[:])
        onesb = const_pool.tile([1, 128], BF16, tag="onesb")
        nc.vector.memset(onesb[:], 1.0)
        ones_col_b = const_pool.tile([128, 1], BF16, tag="onescolb")
        nc.vector.memset(ones_col_b[:], 1.0)
        # broadcast w2 across partitions via K=1 matmul
        W2b = const_pool.tile([128, H], F32, tag="w2b")
        pw = p_mm.tile([128, 512], F32, tag="mm", bufs=4)
        nc.tensor.matmul(pw[:], ones_row[:], wrow[:, H:2 * H],
                         start=True, stop=True)
        nc.vector.tensor_copy(W2b[:], pw[:])

        consts = (ident, identB, ones_row, onesb, ones_col,
                  ones_col_b, w3c, w1c, w1cB, W2b)
        pools = (c_pool, ct_pool, q_pool, pt_pool, sm_pool, scr_pool, a_pool,
                 ot_pool, p_mm, p_sm, p_ty)
        aps = (C_ap, Q_ap, M_ap, O_ap, consts)

        exs = [(rep, b) for rep in range(n_rep)
               for b in range(B_PER_CORE)]
        qstate = {}
        loaded = {}
        PF = 2  # examples of C prefetch ahead of compute
        for i in range(min(PF, len(exs))):
            loaded[i] = _emit_load_ex(nc, pools, aps, i, *exs[i], qstate)
        prev = None
        for i, (rep, b) in enumerate(exs):
            Qall, QallB, mall = qstate[rep]
            st = _emit_prep(nc, pools, aps, b, loaded.pop(i), Qall, QallB,
                            mall)
            j = i + PF
            if j < len(exs):
                loaded[j] = _emit_load_ex(nc, pools, aps, j, *exs[j], qstate)
            if prev is not None:
                _emit_outA(nc, pools, aps, prev)
                _emit_outB(nc, pools, aps, prev)
            prev = st
        _emit_outA(nc, pools, aps, prev)
        _emit_outB(nc, pools, aps, prev)

    nc.compile()
    return nc


_NC_CACHE: dict = {}


def _get_nc(n_rep: int = 1):
    key = ("nc", n_rep)
    if key not in _NC_CACHE:
        _NC_CACHE[key] = build_nc(n_rep)
    return _NC_CACHE[key]


def make_in_maps(C, Q, q_mask, w):
    ident = np.eye(128, dtype=np.float32)
    w = np.ascontiguousarray(w, dtype=np.float32)
    in_maps = []
    for c in range(N_CORES):
        sl = slice(c * B_PER_CORE, (c + 1) * B_PER_CORE)
        in_maps.append({
            "C": np.ascontiguousarray(C[sl], dtype=np.float32),
            "Q": np.ascontiguousarray(Q[sl], dtype=np.float32),
            "q_mask": np.ascontiguousarray(q_mask[sl], dtype=np.int32),
            "w": w,
            "ident": ident,
        })
    return in_maps


def kernel(C, Q, q_mask, w):
    nc = _get_nc(1)
    in_maps = make_in_maps(C, Q, q_mask, w)
    res = run_bass_kernel_spmd(nc, in_maps, list(range(N_CORES)))
    out = np.concatenate([res.results[c]["out"] for c in range(N_CORES)],
                         axis=0)
    return out
